# revision 34
# baseline (speedup 1.0000x reference)
"""Distributed GATv2 (2-layer + BN/MLP) Bass kernel for 8 Trainium2 NeuronCores.

Self-contained: host-side graph partitioning/weight-folding + Bass/Tile device
program + SPMD run + output assembly.

Algorithm notes (validated against reference in numpy to ~1e-3 of absmax):
- Nodes (in-degree sorted, round-robin dealt) -> 8 cores x 3200 slots
  (3125 real + 75 pad); per-core 25 tiles of 128 dst nodes; per tile a
  degree-grid of K_t edge slots per node (K_t identical across cores).
- Per layer, each core computes the full fp16 table
  xl_ext[n] = [SCALE*w ⊙ (x@Wl)[n] | SCALE*c1*(att_h.(x@Wl)_h) | 0-pad]  (512 cols)
  (w = att weights folded with sign into Wl columns) and gathers rows by edge
  slot via dma_gather.  Z = xl_ext[src] + xr_ext[dst] (xr broadcast over k).
- score*SCALE = Z_lin[h] + sum_d (c2*sign(w_d))*|Z_d|  (leaky_relu identity:
  sum w*lrelu(z) = c1*sum(w*z) + c2*sum(sign(w)*|w*z|)).
- ex = exp(score + SHIFT) unnormalized; out = (sum_k ex*Z)/sum_k ex - xr
  (valid since sum alpha = 1), accumulated on the PE via identity-matmuls of
  ex-scaled values; per-column factor SCALE*w undone inside W1/W2 on host.
- b1/b2/bc1/bc2 vanish inside BatchNorm (constant rows).  BN stats via
  channel-major matmuls + AllReduce; h AllGather between layers.

Transfer layout (the axon tunnel ~33 MB/s is the bottleneck, so the host
ships the minimum): per core ONE [128, 3712] fp16 blob = [x_own shard
(3200 cols) | 1/8 weight-payload slice (512 cols)] plus the [16, tot/16]
int16 edge-gather index grid.  The device AllGathers the blobs so every
core sees all x chunks and the full weight payload; replicated row
vectors (biases, att signs, BN params) are broadcast/transposed on the
PE from single rows; the output returns as fp16; output zero-buffers are
created on-device and donated.  The jit(shard_map) dispatch is built
once and cached.
"""
import numpy as np

N = 25000
E = 400000
D = 128
H = 3
HD = H * D
ROW = 512
NEG_SLOPE = 0.2
BN_EPS = 1e-5
NCORES = 8
PER_CORE = 3200
NTILES = 25
NPAD = NCORES * PER_CORE
SCALE = 256.0
EXP_SHIFT = -8.0
C1 = (1.0 + NEG_SLOPE) / 2.0
C2 = (1.0 - NEG_SLOPE) / 2.0
SENT_LIN = -30000.0
P = 128
WS = 512                      # weight-share columns appended to each x shard
QF = 126.45                   # int8 output quant factor (overflow-safe)
XP = 2400                     # packed-x columns (12-bit: 3200 hi8 + 1600 nib)

_BUILD_CACHE = {}
_RUNNER_CACHE = {}


# ----------------------------------------------------------------- host prep
def _build_partition(edge_index):
    src = np.asarray(edge_index[0], np.int64)
    dst = np.asarray(edge_index[1], np.int64)
    deg = np.bincount(dst, minlength=N) + 1
    order = np.argsort(-deg, kind="stable")

    perm = np.full(NPAD, -1, dtype=np.int64)
    node2slot = np.empty(N, dtype=np.int64)
    for c in range(NCORES):
        nodes_c = order[c::NCORES]
        slots = c * PER_CORE + np.arange(len(nodes_c))
        perm[slots] = nodes_c
        node2slot[nodes_c] = slots

    deg_pad = np.ones(NPAD, dtype=np.int64)
    real = perm >= 0
    deg_pad[real] = deg[perm[real]]
    dp = deg_pad.reshape(NCORES, NTILES, 128)
    K = dp.max(axis=(0, 2))
    off_t = np.concatenate([[0], np.cumsum(K * 128)]).astype(np.int64)
    tot_slots = int(off_t[-1])

    SENT = NPAD
    idx = np.full((NCORES, tot_slots), SENT, dtype=np.int32)
    src_slot = node2slot[src]
    dst_slot = node2slot[dst]
    o = np.argsort(dst_slot, kind="stable")
    ss, ds_ = src_slot[o], dst_slot[o]
    gs = np.searchsorted(ds_, np.arange(NPAD), side="left")
    # edge k-position within its dst group (self loop appended at k=deg-1)
    kpos = np.arange(len(ds_)) - gs[ds_]
    all_dst = np.concatenate([ds_, np.arange(NPAD)])           # + self loops
    all_src = np.concatenate([ss, np.arange(NPAD)])
    all_k = np.concatenate([kpos, deg_pad - 1])
    cc, local = np.divmod(all_dst, PER_CORE)
    tt, pp = np.divmod(local, 128)
    flat = off_t[tt] + all_k * 128 + pp
    idx[cc, flat] = all_src
    return dict(perm=perm, K=K, idx=idx, off_t=off_t, tot_slots=tot_slots)


def _fold_weights(inputs):
    out = {}
    for layer, (wl, bl, wr, br, att) in enumerate(
        [(inputs["Wl1"], inputs["bl1"], inputs["Wr1"], inputs["br1"], inputs["att1"]),
         (inputs["Wl2"], inputs["bl2"], inputs["Wr2"], inputs["br2"], inputs["att2"])], 1):
        wl = np.asarray(wl, np.float32); bl = np.asarray(bl, np.float32)
        wr = np.asarray(wr, np.float32); br = np.asarray(br, np.float32)
        att = np.asarray(att, np.float32)
        w = att.reshape(HD)
        Din = wl.shape[0]
        wl_ext = np.zeros((Din, ROW), np.float32)
        wr_ext = np.zeros((Din, ROW), np.float32)
        bias_ext = np.zeros(ROW, np.float32)
        wl_ext[:, :HD] = wl * (SCALE * w)[None, :]
        wr_ext[:, :HD] = wr * (SCALE * w)[None, :]
        for h in range(H):
            cols = slice(h * D, (h + 1) * D)
            wl_ext[:, HD + h] = C1 * SCALE * (wl[:, cols] @ w[cols])
            wr_ext[:, HD + h] = C1 * SCALE * (wr[:, cols] @ w[cols])
        bias_ext[:HD] = (bl + br) * (SCALE * w)
        for h in range(H):
            cols = slice(h * D, (h + 1) * D)
            bias_ext[HD + h] = C1 * SCALE * ((bl[cols] + br[cols]) @ w[cols])
        out[f"wl_ext{layer}"] = wl_ext
        out[f"wr_ext{layer}"] = wr_ext
        out[f"bias_ext{layer}"] = bias_ext
        out[f"sgn{layer}"] = (C2 * np.sign(w)).astype(np.float32)
        out[f"wscale{layer}"] = SCALE * w
    out["W1_eff"] = np.asarray(inputs["W1"], np.float32) / out["wscale1"][:, None]
    W2 = np.asarray(inputs["W2"], np.float32).copy()
    W2[:HD] = W2[:HD] / out["wscale2"][:, None]
    W2[HD:] = W2[HD:] / out["wscale1"][:, None]
    out["W2_eff"] = W2
    return out


def _wrap_idx(idx_core):
    """[tot_slots] int32 -> [16, tot_slots//16] int16 (16-wrapped)."""
    return idx_core.reshape(-1, 16).T.astype(np.int16)


def _weight_shares(fw, inputs, xscale):
    """Pack the replicated weight payload into 8 [128, WS] fp16 shares.
    Share c rides in core c's blob; AllGather reconstructs all of them.
    xscale (the 12-bit x dequant step) is folded into the layer-1 W."""
    f16 = np.float16
    shares = np.zeros((NCORES, P, WS), f16)
    shares[0] = (fw["wl_ext1"] * xscale).astype(f16)
    shares[1] = (fw["wr_ext1"] * xscale).astype(f16)
    shares[2] = fw["wl_ext2"].astype(f16)
    shares[3] = fw["wr_ext2"].astype(f16)
    W1c = fw["W1_eff"].reshape(3, P, P)
    shares[4][:, 0:384] = W1c.transpose(1, 0, 2).reshape(P, 384).astype(f16)
    shares[4][:, 384:512] = np.eye(P, dtype=f16)
    W2c = fw["W2_eff"].reshape(6, P, P)
    shares[5] = W2c[0:4].transpose(1, 0, 2).reshape(P, 512).astype(f16)
    shares[6][:, 0:256] = W2c[4:6].transpose(1, 0, 2).reshape(P, 256).astype(f16)
    # rows: p0 bias1, p1 bias2, p2 sgn1, p3 sgn2, p4 [g1|be1|g2|be2]
    shares[7][0, :] = fw["bias_ext1"].astype(f16)
    shares[7][1, :] = fw["bias_ext2"].astype(f16)
    shares[7][2, 0:HD] = fw["sgn1"].astype(f16)
    shares[7][3, 0:HD] = fw["sgn2"].astype(f16)
    shares[7][4, :] = np.concatenate(
        [np.asarray(inputs["g1"], np.float32), np.asarray(inputs["be1"], np.float32),
         np.asarray(inputs["g2"], np.float32), np.asarray(inputs["be2"], np.float32)]
    ).astype(f16)
    return shares


# ------------------------------------------------------------- device build
def _build_program(K_tuple):
    import concourse.bass as bass
    import concourse.mybir as mybir
    import concourse.tile as tile
    from concourse import bacc

    K = list(K_tuple)
    off_t = np.concatenate([[0], np.cumsum(np.array(K) * 128)]).astype(np.int64)
    tot_slots = int(off_t[-1])
    KMAX = max(K)
    IDXC = tot_slots // 128          # idx columns inside the blob (f16-sized)
    BLOBW = XP + WS + IDXC
    f16, f32, i16 = mybir.dt.float16, mybir.dt.float32, mybir.dt.int16
    i8, u8 = mybir.dt.int8, mybir.dt.uint8
    AF = mybir.ActivationFunctionType
    OP = mybir.AluOpType

    nc = bacc.Bacc("TRN2", target_bir_lowering=False, debug=False,
                   num_devices=NCORES)

    def const_col(val, dtype=f32):
        t = nc.alloc_sbuf_tensor(f"cc-{val}", [P, 1], dtype)
        nc.gpsimd.memset(t.ap(), float(val))
        nc.const_aps.aps[(dtype, float(val))] = t.ap()
        return t.ap()

    shift_ap = const_col(EXP_SHIFT)
    eps_ap = const_col(BN_EPS)
    nc.all_engine_barrier()

    # ---- inputs
    def din(name, shape, dt):
        return nc.dram_tensor(name, shape, dt, kind="ExternalInput")

    t_xw = din("xw", [P, BLOBW], i16)        # [packed x_own | w share | idx]
    t_out = nc.dram_tensor("outT", [P, PER_CORE + 4], i8, kind="ExternalOutput")

    with tile.TileContext(nc) as tc:
        with tc.tile_pool(name="sb", bufs=1) as sb, \
             tc.tile_pool(name="sbB", bufs=2) as sbB, \
             tc.tile_pool(name="sbB3", bufs=2) as sbB3, \
             tc.tile_pool(name="junkp", bufs=4) as junkp, \
             tc.tile_pool(name="psum", bufs=2, space="PSUM") as psp, \
             tc.tile_pool(name="psumD", bufs=4, space="PSUM") as pspD, \
             tc.tile_pool(name="dram", bufs=1, space="DRAM") as dram:

            # dram scratch
            xw_bounce = dram.tile([P, BLOBW], i16, tag="xwb")
            xw_all = dram.tile([NCORES, P, BLOBW], i16, tag="xwall")
            xl_tab = dram.tile([NPAD + P, ROW], f16, tag="xl_tab")
            xin_dram = dram.tile([PER_CORE, HD], f16, tag="xin")
            h2_dram = dram.tile([PER_CORE, HD], f16, tag="h2")
            hT_bounce = dram.tile([P, PER_CORE], f16, tag="hTb")
            hT_all = dram.tile([NCORES, P, PER_CORE], f16, tag="hTall")
            st_in = dram.tile([P, 2], f32, tag="st_in")
            st_out = dram.tile([P, 2], f32, tag="st_out")

            # ---- AllGather the blobs (bounce via SBUF into internal DRAM)
            xw_sb = sbB.tile([P, BLOBW], i16, tag="xwsb", bufs=1)
            nc.sync.dma_start(xw_sb[:], t_xw.ap())
            nc.sync.dma_start(xw_bounce[:], xw_sb[:])
            nc.gpsimd.collective_compute(
                "AllGather", mybir.AluOpType.bypass,
                replica_groups=[list(range(NCORES))],
                ins=[xw_bounce[:].opt()], outs=[xw_all[:].opt()])

            # ---- resident small tensors
            idx_sb = sb.tile([P, tot_slots // 16], i16, tag="idx")
            idx_src = (t_xw.ap()[:, XP + WS:XP + WS + IDXC]
                       .rearrange("(s a) c -> s a c", a=8))
            for g in range(8):
                nc.sync.dma_start(
                    idx_sb[16 * g:16 * (g + 1), :]
                    .rearrange("p (a c) -> p a c", a=8),
                    idx_src)
            ones_row = sb.tile([1, P], f16, tag="ones_row")
            nc.gpsimd.memset(ones_row[:], 1.0)
            rows_sb = []
            for r in range(5):
                row_r = sb.tile([1, ROW], f16, tag=f"row{r}", name=f"row{r}")
                rows_sb.append(row_r)
            for r in range(5):
                nc.sync.dma_start(rows_sb[r][:],
                                  xw_all[7][r:r + 1, XP:XP + ROW].bitcast(f16))
            I_sb = sb.tile([P, P], f16, tag="ident")
            nc.sync.dma_start(I_sb[:],
                              xw_all[4][:, XP + 384:XP + 512].bitcast(f16))
            wl_sb = sb.tile([P, ROW], f16, tag="wl")
            wr_sb = sb.tile([P, ROW], f16, tag="wr")
            bias_sb = sb.tile([P, ROW], f16, tag="bias")
            sgn_sb = sb.tile([P, HD], f16, tag="sgn")
            xr_all = sb.tile([P, NTILES * ROW], f16, tag="xr_all")
            bnp = sb.tile([P, 2], f32, tag="bnp")

            def bcast_row(dst, row_ap, ncols):
                """dst[:, 0:ncols] (f16 sbuf) = broadcast of row_ap [1, ncols]."""
                ps = pspD.tile([P, ROW], f32, tag="psD")
                nc.tensor.matmul(ps[:, 0:ncols], ones_row[:], row_ap,
                                 start=True, stop=True)
                nc.vector.tensor_copy(dst[:, 0:ncols], ps[:, 0:ncols])

            def transpose_row(dst_col, row_ap):
                """dst_col [P, 1] f32 sbuf = row_ap [1, P] transposed."""
                ps = pspD.tile([P, ROW], f32, tag="psD")
                nc.tensor.matmul(ps[:, 0:1], row_ap, ones_row[:, 0:1],
                                 start=True, stop=True)
                nc.vector.tensor_copy(dst_col, ps[:, 0:1])

            def unpack_x(fc, src_ap):
                """fc [P, PER_CORE] f16 <- 12-bit-packed x ints from src_ap
                ([P, XP] f16-typed region: 3200 hi-bytes then 1600 nibble
                pairs).  Values come out as exact integers in [-2047, 2047];
                the dequant step is folded into the layer-1 weights.
                Note: the tile dep-tracker misses reads through size-changing
                bitcast APs, so stage is a native-u8 tile and the one i8 view
                read is sandwiched between tracked native reads on the DVE."""
                stage = sbB.tile([P, 2 * XP], u8, tag="xstage", bufs=1)
                nc.sync.dma_start(stage[:], src_ap.bitcast(u8))
                hi = stage[:, 0:PER_CORE].bitcast(i8)            # [P, 3200]
                nib = stage[:, PER_CORE:2 * XP]                  # [P, 1600] u8
                nl = sbB3.tile([P, PER_CORE // 2], u8, tag="nl", bufs=1)
                nh = sbB3.tile([P, PER_CORE // 2], u8, tag="nh", bufs=1)
                nc.vector.tensor_scalar(out=nl[:], in0=nib, scalar1=15,
                                        scalar2=None, op0=OP.bitwise_and)
                nc.vector.tensor_scalar(out=fc[:], in0=hi, scalar1=16.0,
                                        scalar2=None, op0=OP.mult)
                nc.vector.tensor_scalar(out=nh[:], in0=nib, scalar1=4,
                                        scalar2=None,
                                        op0=OP.logical_shift_right)
                nlf = sbB3.tile([P, PER_CORE // 2], f16, tag="nlf", bufs=1)
                nhf = sbB3.tile([P, PER_CORE // 2], f16, tag="nhf", bufs=1)
                nc.vector.tensor_copy(nlf[:], nl[:])
                nc.vector.tensor_copy(nhf[:], nh[:])
                fcv = fc[:].rearrange("p (c t) -> p c t", t=2)
                nc.vector.tensor_tensor(out=fcv[:, :, 0:1], in0=fcv[:, :, 0:1],
                                        in1=nlf[:, :, None], op=OP.add)
                nc.vector.tensor_tensor(out=fcv[:, :, 1:2], in0=fcv[:, :, 1:2],
                                        in1=nhf[:, :, None], op=OP.add)

            def dense_tables(layer, chunk_src, own_src):
                """Write xl table (all nodes) + xr_all (own shard) for layer.
                chunk_src(c)/own_src() -> DRAM AP for node chunk c / own
                shard: packed [P, XP] for layer 0, plain [P, PER_CORE] f16
                for layer 1."""
                nc.sync.dma_start(wl_sb[:],
                                  xw_all[2 * layer][:, XP:XP + ROW].bitcast(f16))
                nc.sync.dma_start(wr_sb[:],
                                  xw_all[2 * layer + 1][:, XP:XP + ROW].bitcast(f16))
                bcast_row(bias_sb, rows_sb[layer][:], ROW)
                bcast_row(sgn_sb, rows_sb[2 + layer][:, 0:HD], HD)
                for c in range(NCORES):
                    fc = sbB.tile([P, PER_CORE], f16, tag="featchunk")
                    if layer == 0:
                        unpack_x(fc, chunk_src(c))
                    else:
                        nc.sync.dma_start(fc[:], chunk_src(c))
                    for tt in range(NTILES):
                        t = c * NTILES + tt
                        ps = pspD.tile([P, ROW], f32, tag="psD")
                        nc.tensor.matmul(ps[:], fc[:, tt * P:(tt + 1) * P],
                                         wl_sb[:], start=True, stop=True)
                        ot = sbB3.tile([P, ROW], f16, tag="xlrow")
                        if t % 2 == 0:
                            nc.scalar.copy(ot[:], ps[:])
                        else:
                            nc.vector.tensor_copy(ot[:], ps[:])
                        nc.sync.dma_start(xl_tab[t * P:(t + 1) * P, :], ot[:])
                # sentinel rows: zeros except big-negative linear-score cols
                sent_sb = sbB.tile([P, ROW], f16, tag="sentsb")
                nc.gpsimd.memset(sent_sb[:], 0.0)
                nc.gpsimd.memset(sent_sb[:, HD:HD + H], SENT_LIN)
                nc.sync.dma_start(xl_tab[NPAD:NPAD + P, :], sent_sb[:])
                oc = sbB.tile([P, PER_CORE], f16, tag="featchunk")
                if layer == 0:
                    unpack_x(oc, own_src())
                else:
                    nc.sync.dma_start(oc[:], own_src())
                for t in range(NTILES):
                    ps = pspD.tile([P, ROW], f32, tag="psD")
                    nc.tensor.matmul(ps[:], oc[:, t * P:(t + 1) * P],
                                     wr_sb[:], start=True, stop=True)
                    nc.vector.tensor_tensor(
                        out=xr_all[:, t * ROW:(t + 1) * ROW],
                        in0=ps[:], in1=bias_sb[:], op=OP.add)

            def edge_phase(layer, out_dram):
                for t in range(NTILES):
                    kt = K[t]
                    gb = sbB.tile([P, KMAX, ROW], f16, tag="gbufA", bufs=1)
                    o16 = int(off_t[t]) // 16
                    for kc in range(0, kt, 8):
                        nk = min(8, kt - kc)
                        nc.gpsimd.dma_gather(
                            out_ap=gb[:, kc:kc + nk, :],
                            in_ap=xl_tab[:],
                            idxs_ap=idx_sb[:, o16 + kc * 8:o16 + (kc + nk) * 8],
                            num_idxs=nk * P,
                            num_idxs_reg=nk * P,
                            elem_size=ROW,
                        )
                    xr_t = xr_all[:, t * ROW:t * ROW + 388]
                    nc.vector.tensor_tensor(
                        out=gb[:, 0:kt, 0:388], in0=gb[:, 0:kt, 0:388],
                        in1=xr_t[:, None, :].to_broadcast([P, kt, 388]),
                        op=OP.add)
                    sacc = sbB.tile([P, KMAX, 4], f32, tag="sacc")
                    for k in range(kt):
                        ab = sbB3.tile([P, HD], f16, tag="abs")
                        nc.scalar.activation(ab[:], gb[:, k, 0:HD], AF.Abs)
                        for h in range(H):
                            jt = junkp.tile([P, P], f16, tag="junk")
                            nc.vector.scalar_tensor_tensor(
                                out=jt[:],
                                in0=ab[:, h * P:(h + 1) * P],
                                scalar=1.0,
                                in1=sgn_sb[:, h * P:(h + 1) * P],
                                op0=OP.mult, op1=OP.mult,
                                accum_out=sacc[:, k, h:h + 1])
                    nc.vector.tensor_tensor(
                        out=sacc[:, 0:kt, 0:3], in0=sacc[:, 0:kt, 0:3],
                        in1=gb[:, 0:kt, HD:HD + 3], op=OP.add)
                    ex = sbB.tile([P, KMAX, 4], f32, tag="ex")
                    nc.scalar.activation(ex[:, 0:kt, 0:3], sacc[:, 0:kt, 0:3],
                                         AF.Exp, bias=shift_ap,
                                         scale=1.0 / SCALE)
                    den = sbB.tile([P, 4], f32, tag="den")
                    nc.vector.tensor_reduce(
                        out=den[:, 0:3],
                        in_=ex[:, 0:kt, 0:3].rearrange("p k h -> p h k"),
                        axis=mybir.AxisListType.X, op=OP.add)
                    denr = sbB.tile([P, 4], f32, tag="denr")
                    nc.vector.reciprocal(denr[:, 0:3], den[:, 0:3])
                    po = psp.tile([P, HD], f32, tag="pout")
                    for k in range(kt):
                        xls = sbB3.tile([P, HD], f16, tag="xls")
                        for h in range(H):
                            nc.vector.tensor_scalar(
                                out=xls[:, h * P:(h + 1) * P],
                                in0=gb[:, k, h * P:(h + 1) * P],
                                scalar1=ex[:, k, h:h + 1], scalar2=None,
                                op0=OP.mult)
                        nc.tensor.matmul(po[:], I_sb[:], xls[:],
                                         start=(k == 0), stop=(k == kt - 1))
                    xo = sbB3.tile([P, HD], f16, tag="xout")
                    for h in range(H):
                        nc.vector.scalar_tensor_tensor(
                            out=xo[:, h * P:(h + 1) * P],
                            in0=po[:, h * P:(h + 1) * P],
                            scalar=denr[:, h:h + 1],
                            in1=xr_all[:, t * ROW + h * P:t * ROW + (h + 1) * P],
                            op0=OP.mult, op1=OP.subtract)
                    nc.sync.dma_start(out_dram[t * P:(t + 1) * P, :], xo[:])

            def transpose_load(dst_sb, src_dram):
                for c3 in range(3):
                    nc.sync.dma_start_transpose(
                        dst_sb[:, c3 * PER_CORE:(c3 + 1) * PER_CORE],
                        src_dram[:, c3 * P:(c3 + 1) * P])

            def bn_phase(yT, wc_srcs, rhs_list, layer, out_sb):
                """yT [P, PER_CORE] f32 <- sum_chunks Wc.T @ rhs; BN + relu."""
                nchunks = len(wc_srcs)
                Wc_sb = sb.tile([P, nchunks, P], f16, tag=f"wc{nchunks}")
                for kk in range(nchunks):
                    nc.sync.dma_start(Wc_sb[:, kk, :], wc_srcs[kk])
                NCH = (PER_CORE + 511) // 512
                for nci in range(NCH):
                    n0 = nci * 512
                    n1 = min(PER_CORE, n0 + 512)
                    ps = pspD.tile([P, 512], f32, tag="psD")
                    for kk in range(nchunks):
                        rhs = rhs_list[kk]
                        nc.tensor.matmul(ps[:, 0:n1 - n0],
                                         Wc_sb[:, kk, :],
                                         rhs[:, n0:n1],
                                         start=(kk == 0), stop=(kk == nchunks - 1))
                    if nci % 2 == 0:
                        nc.scalar.copy(yT[:, n0:n1], ps[:, 0:n1 - n0])
                    else:
                        nc.vector.tensor_copy(yT[:, n0:n1], ps[:, 0:n1 - n0])
                nc.gpsimd.memset(yT[:, PER_CORE - 75:], 0.0)
                ssum = sbB.tile([P, 2], f32, tag="ssum")
                nc.vector.tensor_reduce(out=ssum[:, 0:1], in_=yT[:],
                                        axis=mybir.AxisListType.X, op=OP.add)
                sqj = sb.tile([P, 3 * PER_CORE], f16, tag="h2T")
                nc.scalar.activation(sqj[:, 0:PER_CORE], yT[:], AF.Square,
                                     accum_out=ssum[:, 1:2])
                nc.sync.dma_start(st_in[:], ssum[:])
                nc.gpsimd.collective_compute(
                    "AllReduce", OP.add,
                    replica_groups=[list(range(NCORES))],
                    ins=[st_in[:].opt()], outs=[st_out[:].opt()])
                stats = sbB.tile([P, 2], f32, tag="stats")
                nc.sync.dma_start(stats[:], st_out[:])
                transpose_row(bnp[:, 0:1],
                              rows_sb[4][:, 256 * layer:256 * layer + P])
                transpose_row(bnp[:, 1:2],
                              rows_sb[4][:, 256 * layer + P:256 * layer + 2 * P])
                mu = sbB.tile([P, 8], f32, tag="mu")
                nc.vector.tensor_scalar(out=mu[:, 0:1], in0=stats[:, 0:1],
                                        scalar1=1.0 / N, scalar2=None, op0=OP.mult)
                nc.vector.tensor_scalar(out=mu[:, 1:2], in0=stats[:, 1:2],
                                        scalar1=1.0 / N, scalar2=None, op0=OP.mult)
                # var = E[y^2] - mu^2: compute (mu*-mu) + E[y2]
                nc.vector.tensor_scalar(out=mu[:, 6:7], in0=mu[:, 0:1],
                                        scalar1=-1.0, scalar2=None, op0=OP.mult)
                nc.vector.scalar_tensor_tensor(
                    out=mu[:, 2:3], in0=mu[:, 0:1], scalar=mu[:, 6:7],
                    in1=mu[:, 1:2], op0=OP.mult, op1=OP.add)
                sd = sbB.tile([P, 2], f32, tag="sd")
                nc.scalar.activation(sd[:, 0:1], mu[:, 2:3], AF.Sqrt, bias=eps_ap)
                nc.vector.reciprocal(sd[:, 1:2], sd[:, 0:1])
                # a = gamma*rs ; b = beta - mu*a
                nc.vector.tensor_tensor(out=mu[:, 3:4], in0=bnp[:, 0:1],
                                        in1=sd[:, 1:2], op=OP.mult)
                nc.vector.scalar_tensor_tensor(
                    out=mu[:, 4:5], in0=mu[:, 0:1], scalar=mu[:, 3:4],
                    in1=bnp[:, 1:2], op0=OP.mult, op1=OP.subtract)
                nc.vector.tensor_scalar(out=mu[:, 5:6], in0=mu[:, 4:5],
                                        scalar1=-1.0, scalar2=None, op0=OP.mult)
                nc.scalar.activation(out_sb[:], yT[:],
                                     AF.Relu, bias=mu[:, 5:6], scale=mu[:, 3:4])

            # ---------------- phase L1 dense
            dense_tables(0,
                         lambda c: xw_all[c][:, 0:XP],
                         lambda: t_xw.ap()[:, 0:XP])
            # ---------------- L1 edge
            edge_phase(0, xin_dram)
            # ---------------- W1 + BN1 + relu -> hT
            xinT_sb = sb.tile([P, 3 * PER_CORE], f16, tag="xinT")
            transpose_load(xinT_sb, xin_dram)
            yT = sb.tile([P, PER_CORE], f32, tag="yT")
            hT_sb = sbB.tile([P, PER_CORE], f16, tag="featchunk")
            bn_phase(yT,
                     [xw_all[4][:, XP + kk * P:XP + (kk + 1) * P].bitcast(f16)
                      for kk in range(3)],
                     [xinT_sb[:, i * PER_CORE:(i + 1) * PER_CORE]
                      for i in range(3)],
                     0, hT_sb)
            nc.sync.dma_start(hT_bounce[:], hT_sb[:])
            nc.gpsimd.collective_compute(
                "AllGather", mybir.AluOpType.bypass,
                replica_groups=[list(range(NCORES))],
                ins=[hT_bounce[:].opt()], outs=[hT_all[:].opt()])
            # ---------------- L2 dense
            dense_tables(1,
                         lambda c: hT_all[c],
                         lambda: hT_bounce[:])
            # ---------------- L2 edge
            edge_phase(1, h2_dram)
            # ---------------- final: W2 on [h2 | x_in] + BN2 + relu
            h2T_sb = sb.tile([P, 3 * PER_CORE], f16, tag="h2T")
            transpose_load(h2T_sb, h2_dram)
            y2T = sb.tile([P, PER_CORE], f32, tag="yT")
            o16_sb = sbB.tile([P, PER_CORE], f16, tag="o16", bufs=1)
            w2_srcs = ([xw_all[5][:, XP + kk * P:XP + (kk + 1) * P].bitcast(f16)
                        for kk in range(4)] +
                       [xw_all[6][:, XP + kk * P:XP + (kk + 1) * P].bitcast(f16)
                        for kk in range(2)])
            bn_phase(y2T, w2_srcs,
                     [h2T_sb[:, i * PER_CORE:(i + 1) * PER_CORE]
                      for i in range(3)] +
                     [xinT_sb[:, i * PER_CORE:(i + 1) * PER_CORE]
                      for i in range(3)],
                     1, o16_sb)
            # int8 quantization with per-partition (=channel) scale
            rmax = sbB.tile([P, 4], f32, tag="rmax")
            nc.vector.tensor_reduce(out=rmax[:, 0:1], in_=o16_sb[:],
                                    axis=mybir.AxisListType.X, op=OP.max)
            nc.vector.tensor_scalar(out=rmax[:, 1:2], in0=rmax[:, 0:1],
                                    scalar1=1e-6, scalar2=None, op0=OP.max)
            nc.vector.reciprocal(rmax[:, 2:3], rmax[:, 1:2])
            nc.vector.tensor_scalar(out=rmax[:, 3:4], in0=rmax[:, 2:3],
                                    scalar1=QF, scalar2=None, op0=OP.mult)
            qt = sbB.tile([P, PER_CORE + 4], i8, tag="qt", bufs=1)
            nc.vector.tensor_scalar(out=qt[:, 0:PER_CORE], in0=o16_sb[:],
                                    scalar1=rmax[:, 3:4], scalar2=None,
                                    op0=OP.mult)
            sc = sbB.tile([P, 1], f32, tag="sc")
            nc.vector.tensor_scalar(out=sc[:], in0=rmax[:, 1:2],
                                    scalar1=1.0 / QF, scalar2=None, op0=OP.mult)
            nc.vector.tensor_copy(qt[:, PER_CORE:PER_CORE + 4],
                                  sc[:].bitcast(i8))
            nc.sync.dma_start(t_out.ap(), qt[:])

    nc.compile()
    return nc


# ------------------------------------------------------------- cached runner
def _build_runner(nc):
    import jax
    import jax.numpy as jnp
    from jax.sharding import Mesh, PartitionSpec, NamedSharding
    from jax.experimental.shard_map import shard_map
    import concourse.mybir as mybir
    from concourse.bass2jax import (_bass_exec_p, partition_id_tensor,
                                    install_neuronx_cc_hook)

    install_neuronx_cc_hook()
    partition_name = (nc.partition_id_tensor.name
                      if nc.partition_id_tensor else None)
    in_names, out_names, out_avals = [], [], []
    for alloc in nc.m.functions[0].allocations:
        if not isinstance(alloc, mybir.MemoryLocationSet):
            continue
        name = alloc.memorylocations[0].name
        if alloc.kind == "ExternalInput":
            if name != partition_name:
                in_names.append(name)
        elif alloc.kind == "ExternalOutput":
            out_avals.append(jax.core.ShapedArray(tuple(alloc.tensor_shape),
                                                  mybir.dt.np(alloc.dtype)))
            out_names.append(name)
    n_params = len(in_names)
    n_outs = len(out_avals)
    in_names_all = in_names + out_names + (
        [partition_name] if partition_name else [])

    def _body(*args):
        operands = list(args)
        if partition_name is not None:
            operands.append(partition_id_tensor())
        return tuple(_bass_exec_p.bind(
            *operands, out_avals=tuple(out_avals),
            in_names=tuple(in_names_all), out_names=tuple(out_names),
            lowering_input_output_aliases=(), sim_require_finite=True,
            sim_require_nnan=True, nc=nc))

    mesh = Mesh(np.asarray(jax.devices()[:NCORES]), ("core",))
    sharding = NamedSharding(mesh, PartitionSpec("core"))
    donate = tuple(range(n_params, n_params + n_outs))
    sharded = jax.jit(
        shard_map(_body, mesh=mesh,
                  in_specs=(PartitionSpec("core"),) * (n_params + n_outs),
                  out_specs=(PartitionSpec("core"),) * n_outs,
                  check_rep=False),
        donate_argnums=donate, keep_unused=True)
    zshapes = [(NCORES * a.shape[0], *a.shape[1:]) for a in out_avals]
    zdtypes = [a.dtype for a in out_avals]
    make_zeros = jax.jit(
        lambda: tuple(jnp.zeros(s, d) for s, d in zip(zshapes, zdtypes)),
        out_shardings=tuple(sharding for _ in zshapes))

    def run(in_map_concat):
        """in_map_concat: name -> concatenated-along-axis0 np array."""
        dev_in = [jax.device_put(in_map_concat[name], sharding)
                  for name in in_names]
        zs = make_zeros()
        out_arrs = sharded(*dev_in, *zs)
        return {name: np.asarray(out_arrs[i])
                for i, name in enumerate(out_names)}

    return run


# ----------------------------------------------------------------- kernel()
def kernel(**inputs):
    part = _build_partition(np.asarray(inputs["edge_index"]))
    fw = _fold_weights(inputs)
    perm, K, idx = part["perm"], part["K"], part["idx"]

    key = tuple(int(k) for k in K)
    if key not in _BUILD_CACHE:
        _BUILD_CACHE[key] = _build_program(key)
    nc = _BUILD_CACHE[key]
    if key not in _RUNNER_CACHE:
        _RUNNER_CACHE[key] = _build_runner(nc)
    run = _RUNNER_CACHE[key]

    x = np.asarray(inputs["x"], np.float32)
    xpad = np.zeros((NPAD, D), np.float32)
    real = perm >= 0
    xpad[real] = x[perm[real]]
    am = max(float(np.abs(x).max()), 1e-30)
    xq = np.round(xpad.T * (2047.0 / am)).astype(np.int16)   # [128, NPAD]
    hi8 = (xq >> 4).astype(np.int8)
    nib = (xq & 15).astype(np.uint8)
    shares = _weight_shares(fw, inputs, am / 2047.0)

    IDXC = part["tot_slots"] // 128
    BLOBW = XP + WS + IDXC
    blob = np.empty((NCORES * P, BLOBW), np.int16)
    for c in range(NCORES):
        b = blob[c * P:(c + 1) * P]
        sl = slice(c * PER_CORE, (c + 1) * PER_CORE)
        nib_c = nib[:, sl]
        packed = np.ascontiguousarray(np.concatenate(
            [hi8[:, sl].view(np.uint8),
             nib_c[:, 0::2] | (nib_c[:, 1::2] << 4)], axis=1))
        b[:, 0:XP] = packed.view(np.int16)
        b[:, XP:XP + WS] = shares[c].view(np.int16)
        b[:, XP + WS:] = _wrap_idx(idx[c]).reshape(P, IDXC)

    import time as _time
    _t0 = _time.time()
    res = run({"xw": blob})
    kernel._last_run_s = _time.time() - _t0

    o = res["outT"].reshape(NCORES, P, PER_CORE + 4)
    q = o[:, :, 0:PER_CORE].astype(np.float32)
    sc = np.ascontiguousarray(o[:, :, PER_CORE:]).view(np.float32)  # [8, P, 1]
    oT = q * sc
    out = np.zeros((N, D), np.float32)
    for c in range(NCORES):
        sl = slice(c * PER_CORE, (c + 1) * PER_CORE)
        rr = real[sl]
        out[perm[sl][rr]] = oT[c].T[rr]
    return out


if __name__ == "__main__":
    import time
    data = np.load("/root/problem/inputs_cache.npy", allow_pickle=True).item()
    expected = np.load("/root/problem/expected_cache.npy")
    t0 = time.time()
    out = kernel(**data)
    print(f"kernel() took {time.time()-t0:.1f}s")
    err = np.abs(out - expected)
    am = np.abs(expected).max()
    print(f"max_abs_err={err.max():.6f} absmax={am:.4f} rel={err.max()/am:.2e}")
    for _ in range(3):
        t0 = time.time()
        kernel(**data)
        print(f"repeat: {time.time()-t0:.2f}s (run {kernel._last_run_s:.3f}s)")


# revision 36
# speedup vs baseline: 1.5810x; 1.5810x over previous
"""Distributed GATv2 (2-layer + BN/MLP) Bass kernel for 8 Trainium2 NeuronCores.

Self-contained: host-side graph partitioning/weight-folding + Bass/Tile device
program + SPMD run + output assembly.

Algorithm notes (validated against reference in numpy to ~1e-3 of absmax):
- Nodes (in-degree sorted, round-robin dealt) -> 8 cores x 3200 slots
  (3125 real + 75 pad); per-core 25 tiles of 128 dst nodes; per tile a
  degree-grid of K_t edge slots per node (K_t identical across cores).
- Per layer, each core computes the full fp16 table
  xl_ext[n] = [SCALE*w ⊙ (x@Wl)[n] | SCALE*c1*(att_h.(x@Wl)_h) | 0-pad]  (512 cols)
  (w = att weights folded with sign into Wl columns) and gathers rows by edge
  slot via dma_gather.  Z = xl_ext[src] + xr_ext[dst] (xr broadcast over k).
- score*SCALE = Z_lin[h] + sum_d (c2*sign(w_d))*|Z_d|  (leaky_relu identity:
  sum w*lrelu(z) = c1*sum(w*z) + c2*sum(sign(w)*|w*z|)).
- ex = exp(score + SHIFT) unnormalized; out = (sum_k ex*Z)/sum_k ex - xr
  (valid since sum alpha = 1), accumulated on the PE via identity-matmuls of
  ex-scaled values; per-column factor SCALE*w undone inside W1/W2 on host.
- b1/b2/bc1/bc2 vanish inside BatchNorm (constant rows).  BN stats via
  channel-major matmuls + AllReduce; h AllGather between layers.

Transfer layout (the axon tunnel ~30-60 MB/s serialized link is the
bottleneck, so the host ships the minimum): per core ONE int16 blob
[128, 2400+512+tot/128] = [12-bit-packed x shard (3200 hi-bytes + 1600
nibble pairs; dequant step folded into the layer-1 weights) | 1/8 of
the fp16 weight payload | the 16-wrapped edge-gather index grid].  The
device AllGathers the blobs (int16 dtype: the collective byte-mangles
non-fp16 bit patterns of REMOTE shards if declared f16) so every core
sees all x chunks and the full weight payload; replicated row vectors
(biases, att signs, BN params) are broadcast/transposed on the PE from
single rows; the output returns as int8 with per-channel scales packed
into 4 trailing bytes per partition; output zero-buffers are created
on-device and donated.  The jit(shard_map) dispatch is built once and
cached.  Fixed axon RPC latency is ~78 ms exec + ~80 ms fetch.
"""
import numpy as np

N = 25000
E = 400000
D = 128
H = 3
HD = H * D
ROW = 512
NEG_SLOPE = 0.2
BN_EPS = 1e-5
NCORES = 8
PER_CORE = 3200
NTILES = 25
NPAD = NCORES * PER_CORE
SCALE = 256.0
EXP_SHIFT = -8.0
C1 = (1.0 + NEG_SLOPE) / 2.0
C2 = (1.0 - NEG_SLOPE) / 2.0
SENT_LIN = -30000.0
P = 128
WS = 512                      # weight-share columns appended to each x shard
QF = 126.45                   # int8 output quant factor (overflow-safe)
XP = 2400                     # packed-x columns (12-bit: 3200 hi8 + 1600 nib)

_BUILD_CACHE = {}
_RUNNER_CACHE = {}


# ----------------------------------------------------------------- host prep
def _build_partition(edge_index):
    src = np.asarray(edge_index[0], np.int64)
    dst = np.asarray(edge_index[1], np.int64)
    deg = np.bincount(dst, minlength=N) + 1
    order = np.argsort(-deg, kind="stable")

    perm = np.full(NPAD, -1, dtype=np.int64)
    node2slot = np.empty(N, dtype=np.int64)
    for c in range(NCORES):
        nodes_c = order[c::NCORES]
        slots = c * PER_CORE + np.arange(len(nodes_c))
        perm[slots] = nodes_c
        node2slot[nodes_c] = slots

    deg_pad = np.ones(NPAD, dtype=np.int64)
    real = perm >= 0
    deg_pad[real] = deg[perm[real]]
    dp = deg_pad.reshape(NCORES, NTILES, 128)
    K = dp.max(axis=(0, 2))
    off_t = np.concatenate([[0], np.cumsum(K * 128)]).astype(np.int64)
    tot_slots = int(off_t[-1])

    SENT = NPAD
    idx = np.full((NCORES, tot_slots), SENT, dtype=np.int32)
    src_slot = node2slot[src]
    dst_slot = node2slot[dst]
    o = np.argsort(dst_slot, kind="stable")
    ss, ds_ = src_slot[o], dst_slot[o]
    gs = np.searchsorted(ds_, np.arange(NPAD), side="left")
    # edge k-position within its dst group (self loop appended at k=deg-1)
    kpos = np.arange(len(ds_)) - gs[ds_]
    all_dst = np.concatenate([ds_, np.arange(NPAD)])           # + self loops
    all_src = np.concatenate([ss, np.arange(NPAD)])
    all_k = np.concatenate([kpos, deg_pad - 1])
    cc, local = np.divmod(all_dst, PER_CORE)
    tt, pp = np.divmod(local, 128)
    flat = off_t[tt] + all_k * 128 + pp
    idx[cc, flat] = all_src
    return dict(perm=perm, K=K, idx=idx, off_t=off_t, tot_slots=tot_slots)


def _fold_weights(inputs):
    out = {}
    for layer, (wl, bl, wr, br, att) in enumerate(
        [(inputs["Wl1"], inputs["bl1"], inputs["Wr1"], inputs["br1"], inputs["att1"]),
         (inputs["Wl2"], inputs["bl2"], inputs["Wr2"], inputs["br2"], inputs["att2"])], 1):
        wl = np.asarray(wl, np.float32); bl = np.asarray(bl, np.float32)
        wr = np.asarray(wr, np.float32); br = np.asarray(br, np.float32)
        att = np.asarray(att, np.float32)
        w = att.reshape(HD)
        Din = wl.shape[0]
        wl_ext = np.zeros((Din, ROW), np.float32)
        wr_ext = np.zeros((Din, ROW), np.float32)
        bias_ext = np.zeros(ROW, np.float32)
        wl_ext[:, :HD] = wl * (SCALE * w)[None, :]
        wr_ext[:, :HD] = wr * (SCALE * w)[None, :]
        for h in range(H):
            cols = slice(h * D, (h + 1) * D)
            wl_ext[:, HD + h] = C1 * SCALE * (wl[:, cols] @ w[cols])
            wr_ext[:, HD + h] = C1 * SCALE * (wr[:, cols] @ w[cols])
        bias_ext[:HD] = (bl + br) * (SCALE * w)
        for h in range(H):
            cols = slice(h * D, (h + 1) * D)
            bias_ext[HD + h] = C1 * SCALE * ((bl[cols] + br[cols]) @ w[cols])
        out[f"wl_ext{layer}"] = wl_ext
        out[f"wr_ext{layer}"] = wr_ext
        out[f"bias_ext{layer}"] = bias_ext
        out[f"sgn{layer}"] = (C2 * np.sign(w)).astype(np.float32)
        out[f"wscale{layer}"] = SCALE * w
    out["W1_eff"] = np.asarray(inputs["W1"], np.float32) / out["wscale1"][:, None]
    W2 = np.asarray(inputs["W2"], np.float32).copy()
    W2[:HD] = W2[:HD] / out["wscale2"][:, None]
    W2[HD:] = W2[HD:] / out["wscale1"][:, None]
    out["W2_eff"] = W2
    return out


def _wrap_idx(idx_core):
    """[tot_slots] int32 -> [16, tot_slots//16] int16 (16-wrapped)."""
    return idx_core.reshape(-1, 16).T.astype(np.int16)


def _weight_shares(fw, inputs, xscale):
    """Pack the replicated weight payload into 8 [128, WS] fp16 shares.
    Share c rides in core c's blob; AllGather reconstructs all of them.
    xscale (the 12-bit x dequant step) is folded into the layer-1 W."""
    f16 = np.float16
    shares = np.zeros((NCORES, P, WS), f16)
    shares[0] = (fw["wl_ext1"] * xscale).astype(f16)
    shares[1] = (fw["wr_ext1"] * xscale).astype(f16)
    shares[2] = fw["wl_ext2"].astype(f16)
    shares[3] = fw["wr_ext2"].astype(f16)
    W1c = fw["W1_eff"].reshape(3, P, P)
    shares[4][:, 0:384] = W1c.transpose(1, 0, 2).reshape(P, 384).astype(f16)
    shares[4][:, 384:512] = np.eye(P, dtype=f16)
    W2c = fw["W2_eff"].reshape(6, P, P)
    shares[5] = W2c[0:4].transpose(1, 0, 2).reshape(P, 512).astype(f16)
    shares[6][:, 0:256] = W2c[4:6].transpose(1, 0, 2).reshape(P, 256).astype(f16)
    # rows: p0 bias1, p1 bias2, p2 sgn1, p3 sgn2, p4 [g1|be1|g2|be2]
    shares[7][0, :] = fw["bias_ext1"].astype(f16)
    shares[7][1, :] = fw["bias_ext2"].astype(f16)
    shares[7][2, 0:HD] = fw["sgn1"].astype(f16)
    shares[7][3, 0:HD] = fw["sgn2"].astype(f16)
    shares[7][4, :] = np.concatenate(
        [np.asarray(inputs["g1"], np.float32), np.asarray(inputs["be1"], np.float32),
         np.asarray(inputs["g2"], np.float32), np.asarray(inputs["be2"], np.float32)]
    ).astype(f16)
    return shares


# ------------------------------------------------------------- device build
def _build_program(K_tuple):
    import concourse.bass as bass
    import concourse.mybir as mybir
    import concourse.tile as tile
    from concourse import bacc

    K = list(K_tuple)
    off_t = np.concatenate([[0], np.cumsum(np.array(K) * 128)]).astype(np.int64)
    tot_slots = int(off_t[-1])
    KMAX = max(K)
    IDXC = tot_slots // 128          # idx columns inside the blob (f16-sized)
    BLOBW = XP + WS + IDXC
    f16, f32, i16 = mybir.dt.float16, mybir.dt.float32, mybir.dt.int16
    i8, u8 = mybir.dt.int8, mybir.dt.uint8
    AF = mybir.ActivationFunctionType
    OP = mybir.AluOpType

    nc = bacc.Bacc("TRN2", target_bir_lowering=False, debug=False,
                   num_devices=NCORES)

    def const_col(val, dtype=f32):
        t = nc.alloc_sbuf_tensor(f"cc-{val}", [P, 1], dtype)
        nc.gpsimd.memset(t.ap(), float(val))
        nc.const_aps.aps[(dtype, float(val))] = t.ap()
        return t.ap()

    shift_ap = const_col(EXP_SHIFT)
    eps_ap = const_col(BN_EPS)
    nc.all_engine_barrier()

    # ---- inputs
    def din(name, shape, dt):
        return nc.dram_tensor(name, shape, dt, kind="ExternalInput")

    t_xw = din("xw", [P, BLOBW], i16)        # [packed x_own | w share | idx]
    t_out = nc.dram_tensor("outT", [P, PER_CORE + 4], i8, kind="ExternalOutput")

    with tile.TileContext(nc) as tc:
        with tc.tile_pool(name="sb", bufs=1) as sb, \
             tc.tile_pool(name="sbB", bufs=2) as sbB, \
             tc.tile_pool(name="sbB3", bufs=2) as sbB3, \
             tc.tile_pool(name="junkp", bufs=4) as junkp, \
             tc.tile_pool(name="psum", bufs=2, space="PSUM") as psp, \
             tc.tile_pool(name="psumD", bufs=4, space="PSUM") as pspD, \
             tc.tile_pool(name="dram", bufs=1, space="DRAM") as dram:

            # dram scratch
            xw_bounce = dram.tile([P, BLOBW], i16, tag="xwb")
            xw_all = dram.tile([NCORES, P, BLOBW], i16, tag="xwall")
            xl_tab = dram.tile([NPAD + P, ROW], f16, tag="xl_tab")
            xin_dram = dram.tile([PER_CORE, HD], f16, tag="xin")
            h2_dram = dram.tile([PER_CORE, HD], f16, tag="h2")
            hT_bounce = dram.tile([P, PER_CORE], f16, tag="hTb")
            hT_all = dram.tile([NCORES, P, PER_CORE], f16, tag="hTall")
            st_in = dram.tile([P, 2], f32, tag="st_in")
            st_out = dram.tile([P, 2], f32, tag="st_out")

            # ---- AllGather the blobs (bounce via SBUF into internal DRAM)
            xw_sb = sbB.tile([P, BLOBW], i16, tag="xwsb", bufs=1)
            nc.sync.dma_start(xw_sb[:], t_xw.ap())
            nc.sync.dma_start(xw_bounce[:], xw_sb[:])
            nc.gpsimd.collective_compute(
                "AllGather", mybir.AluOpType.bypass,
                replica_groups=[list(range(NCORES))],
                ins=[xw_bounce[:].opt()], outs=[xw_all[:].opt()])

            # ---- resident small tensors
            idx_sb = sb.tile([P, tot_slots // 16], i16, tag="idx")
            idx_src = (t_xw.ap()[:, XP + WS:XP + WS + IDXC]
                       .rearrange("(s a) c -> s a c", a=8))
            for g in range(8):
                nc.sync.dma_start(
                    idx_sb[16 * g:16 * (g + 1), :]
                    .rearrange("p (a c) -> p a c", a=8),
                    idx_src)
            ones_row = sb.tile([1, P], f16, tag="ones_row")
            nc.gpsimd.memset(ones_row[:], 1.0)
            rows_sb = []
            for r in range(5):
                row_r = sb.tile([1, ROW], f16, tag=f"row{r}", name=f"row{r}")
                rows_sb.append(row_r)
            for r in range(5):
                nc.sync.dma_start(rows_sb[r][:],
                                  xw_all[7][r:r + 1, XP:XP + ROW].bitcast(f16))
            I_sb = sb.tile([P, P], f16, tag="ident")
            nc.sync.dma_start(I_sb[:],
                              xw_all[4][:, XP + 384:XP + 512].bitcast(f16))
            wl_sb = sb.tile([P, ROW], f16, tag="wl")
            wr_sb = sb.tile([P, ROW], f16, tag="wr")
            bias_sb = sb.tile([P, ROW], f16, tag="bias")
            sgn_sb = sb.tile([P, HD], f16, tag="sgn")
            xr_all = sb.tile([P, NTILES * ROW], f16, tag="xr_all")
            bnp = sb.tile([P, 2], f32, tag="bnp")

            def bcast_row(dst, row_ap, ncols):
                """dst[:, 0:ncols] (f16 sbuf) = broadcast of row_ap [1, ncols]."""
                ps = pspD.tile([P, ROW], f32, tag="psD")
                nc.tensor.matmul(ps[:, 0:ncols], ones_row[:], row_ap,
                                 start=True, stop=True)
                nc.vector.tensor_copy(dst[:, 0:ncols], ps[:, 0:ncols])

            def transpose_row(dst_col, row_ap):
                """dst_col [P, 1] f32 sbuf = row_ap [1, P] transposed."""
                ps = pspD.tile([P, ROW], f32, tag="psD")
                nc.tensor.matmul(ps[:, 0:1], row_ap, ones_row[:, 0:1],
                                 start=True, stop=True)
                nc.vector.tensor_copy(dst_col, ps[:, 0:1])

            def unpack_x(fc, src_ap):
                """fc [P, PER_CORE] f16 <- 12-bit-packed x ints from src_ap
                ([P, XP] f16-typed region: 3200 hi-bytes then 1600 nibble
                pairs).  Values come out as exact integers in [-2047, 2047];
                the dequant step is folded into the layer-1 weights.
                Note: the tile dep-tracker misses reads through size-changing
                bitcast APs, so stage is a native-u8 tile and the one i8 view
                read is sandwiched between tracked native reads on the DVE."""
                stage = sbB.tile([P, 2 * XP], u8, tag="xstage", bufs=1)
                nc.sync.dma_start(stage[:], src_ap.bitcast(u8))
                hi = stage[:, 0:PER_CORE].bitcast(i8)            # [P, 3200]
                nib = stage[:, PER_CORE:2 * XP]                  # [P, 1600] u8
                nl = sbB3.tile([P, PER_CORE // 2], u8, tag="nl", bufs=1)
                nh = sbB3.tile([P, PER_CORE // 2], u8, tag="nh", bufs=1)
                nc.vector.tensor_scalar(out=nl[:], in0=nib, scalar1=15,
                                        scalar2=None, op0=OP.bitwise_and)
                nc.vector.tensor_scalar(out=fc[:], in0=hi, scalar1=16.0,
                                        scalar2=None, op0=OP.mult)
                nc.vector.tensor_scalar(out=nh[:], in0=nib, scalar1=4,
                                        scalar2=None,
                                        op0=OP.logical_shift_right)
                nlf = sbB3.tile([P, PER_CORE // 2], f16, tag="nlf", bufs=1)
                nhf = sbB3.tile([P, PER_CORE // 2], f16, tag="nhf", bufs=1)
                nc.vector.tensor_copy(nlf[:], nl[:])
                nc.vector.tensor_copy(nhf[:], nh[:])
                fcv = fc[:].rearrange("p (c t) -> p c t", t=2)
                nc.vector.tensor_tensor(out=fcv[:, :, 0:1], in0=fcv[:, :, 0:1],
                                        in1=nlf[:, :, None], op=OP.add)
                nc.vector.tensor_tensor(out=fcv[:, :, 1:2], in0=fcv[:, :, 1:2],
                                        in1=nhf[:, :, None], op=OP.add)

            def dense_tables(layer, chunk_src, own_src):
                """Write xl table (all nodes) + xr_all (own shard) for layer.
                chunk_src(c)/own_src() -> DRAM AP for node chunk c / own
                shard: packed [P, XP] for layer 0, plain [P, PER_CORE] f16
                for layer 1."""
                nc.sync.dma_start(wl_sb[:],
                                  xw_all[2 * layer][:, XP:XP + ROW].bitcast(f16))
                nc.sync.dma_start(wr_sb[:],
                                  xw_all[2 * layer + 1][:, XP:XP + ROW].bitcast(f16))
                bcast_row(bias_sb, rows_sb[layer][:], ROW)
                bcast_row(sgn_sb, rows_sb[2 + layer][:, 0:HD], HD)
                for c in range(NCORES):
                    fc = sbB.tile([P, PER_CORE], f16, tag="featchunk")
                    if layer == 0:
                        unpack_x(fc, chunk_src(c))
                    else:
                        nc.sync.dma_start(fc[:], chunk_src(c))
                    for tt in range(NTILES):
                        t = c * NTILES + tt
                        ps = pspD.tile([P, ROW], f32, tag="psD")
                        nc.tensor.matmul(ps[:], fc[:, tt * P:(tt + 1) * P],
                                         wl_sb[:], start=True, stop=True)
                        ot = sbB3.tile([P, ROW], f16, tag="xlrow")
                        if t % 2 == 0:
                            nc.scalar.copy(ot[:], ps[:])
                        else:
                            nc.vector.tensor_copy(ot[:], ps[:])
                        nc.sync.dma_start(xl_tab[t * P:(t + 1) * P, :], ot[:])
                # sentinel rows: zeros except big-negative linear-score cols
                sent_sb = sbB.tile([P, ROW], f16, tag="sentsb")
                nc.gpsimd.memset(sent_sb[:], 0.0)
                nc.gpsimd.memset(sent_sb[:, HD:HD + H], SENT_LIN)
                nc.sync.dma_start(xl_tab[NPAD:NPAD + P, :], sent_sb[:])
                oc = sbB.tile([P, PER_CORE], f16, tag="featchunk")
                if layer == 0:
                    unpack_x(oc, own_src())
                else:
                    nc.sync.dma_start(oc[:], own_src())
                for t in range(NTILES):
                    ps = pspD.tile([P, ROW], f32, tag="psD")
                    nc.tensor.matmul(ps[:], oc[:, t * P:(t + 1) * P],
                                     wr_sb[:], start=True, stop=True)
                    nc.vector.tensor_tensor(
                        out=xr_all[:, t * ROW:(t + 1) * ROW],
                        in0=ps[:], in1=bias_sb[:], op=OP.add)

            def edge_phase(layer, out_dram):
                for t in range(NTILES):
                    kt = K[t]
                    gb = sbB.tile([P, KMAX, ROW], f16, tag="gbufA", bufs=1)
                    o16 = int(off_t[t]) // 16
                    for kc in range(0, kt, 8):
                        nk = min(8, kt - kc)
                        nc.gpsimd.dma_gather(
                            out_ap=gb[:, kc:kc + nk, :],
                            in_ap=xl_tab[:],
                            idxs_ap=idx_sb[:, o16 + kc * 8:o16 + (kc + nk) * 8],
                            num_idxs=nk * P,
                            num_idxs_reg=nk * P,
                            elem_size=ROW,
                        )
                    xr_t = xr_all[:, t * ROW:t * ROW + 388]
                    nc.vector.tensor_tensor(
                        out=gb[:, 0:kt, 0:388], in0=gb[:, 0:kt, 0:388],
                        in1=xr_t[:, None, :].to_broadcast([P, kt, 388]),
                        op=OP.add)
                    sacc = sbB.tile([P, KMAX, 4], f32, tag="sacc")
                    for k in range(kt):
                        ab = sbB3.tile([P, HD], f16, tag="abs")
                        nc.scalar.activation(ab[:], gb[:, k, 0:HD], AF.Abs)
                        for h in range(H):
                            jt = junkp.tile([P, P], f16, tag="junk")
                            nc.vector.scalar_tensor_tensor(
                                out=jt[:],
                                in0=ab[:, h * P:(h + 1) * P],
                                scalar=1.0,
                                in1=sgn_sb[:, h * P:(h + 1) * P],
                                op0=OP.mult, op1=OP.mult,
                                accum_out=sacc[:, k, h:h + 1])
                    nc.vector.tensor_tensor(
                        out=sacc[:, 0:kt, 0:3], in0=sacc[:, 0:kt, 0:3],
                        in1=gb[:, 0:kt, HD:HD + 3], op=OP.add)
                    ex = sbB.tile([P, KMAX, 4], f32, tag="ex")
                    nc.scalar.activation(ex[:, 0:kt, 0:3], sacc[:, 0:kt, 0:3],
                                         AF.Exp, bias=shift_ap,
                                         scale=1.0 / SCALE)
                    den = sbB.tile([P, 4], f32, tag="den")
                    nc.vector.tensor_reduce(
                        out=den[:, 0:3],
                        in_=ex[:, 0:kt, 0:3].rearrange("p k h -> p h k"),
                        axis=mybir.AxisListType.X, op=OP.add)
                    denr = sbB.tile([P, 4], f32, tag="denr")
                    nc.vector.reciprocal(denr[:, 0:3], den[:, 0:3])
                    po = psp.tile([P, HD], f32, tag="pout")
                    for k in range(kt):
                        xls = sbB3.tile([P, HD], f16, tag="xls")
                        for h in range(H):
                            nc.vector.tensor_scalar(
                                out=xls[:, h * P:(h + 1) * P],
                                in0=gb[:, k, h * P:(h + 1) * P],
                                scalar1=ex[:, k, h:h + 1], scalar2=None,
                                op0=OP.mult)
                        nc.tensor.matmul(po[:], I_sb[:], xls[:],
                                         start=(k == 0), stop=(k == kt - 1))
                    xo = sbB3.tile([P, HD], f16, tag="xout")
                    for h in range(H):
                        nc.vector.scalar_tensor_tensor(
                            out=xo[:, h * P:(h + 1) * P],
                            in0=po[:, h * P:(h + 1) * P],
                            scalar=denr[:, h:h + 1],
                            in1=xr_all[:, t * ROW + h * P:t * ROW + (h + 1) * P],
                            op0=OP.mult, op1=OP.subtract)
                    nc.sync.dma_start(out_dram[t * P:(t + 1) * P, :], xo[:])

            def transpose_load(dst_sb, src_dram):
                for c3 in range(3):
                    nc.sync.dma_start_transpose(
                        dst_sb[:, c3 * PER_CORE:(c3 + 1) * PER_CORE],
                        src_dram[:, c3 * P:(c3 + 1) * P])

            def bn_phase(yT, wc_srcs, rhs_list, layer, out_sb):
                """yT [P, PER_CORE] f32 <- sum_chunks Wc.T @ rhs; BN + relu."""
                nchunks = len(wc_srcs)
                Wc_sb = sb.tile([P, nchunks, P], f16, tag=f"wc{nchunks}")
                for kk in range(nchunks):
                    nc.sync.dma_start(Wc_sb[:, kk, :], wc_srcs[kk])
                NCH = (PER_CORE + 511) // 512
                for nci in range(NCH):
                    n0 = nci * 512
                    n1 = min(PER_CORE, n0 + 512)
                    ps = pspD.tile([P, 512], f32, tag="psD")
                    for kk in range(nchunks):
                        rhs = rhs_list[kk]
                        nc.tensor.matmul(ps[:, 0:n1 - n0],
                                         Wc_sb[:, kk, :],
                                         rhs[:, n0:n1],
                                         start=(kk == 0), stop=(kk == nchunks - 1))
                    if nci % 2 == 0:
                        nc.scalar.copy(yT[:, n0:n1], ps[:, 0:n1 - n0])
                    else:
                        nc.vector.tensor_copy(yT[:, n0:n1], ps[:, 0:n1 - n0])
                nc.gpsimd.memset(yT[:, PER_CORE - 75:], 0.0)
                ssum = sbB.tile([P, 2], f32, tag="ssum")
                nc.vector.tensor_reduce(out=ssum[:, 0:1], in_=yT[:],
                                        axis=mybir.AxisListType.X, op=OP.add)
                sqj = sb.tile([P, 3 * PER_CORE], f16, tag="h2T")
                nc.scalar.activation(sqj[:, 0:PER_CORE], yT[:], AF.Square,
                                     accum_out=ssum[:, 1:2])
                nc.sync.dma_start(st_in[:], ssum[:])
                nc.gpsimd.collective_compute(
                    "AllReduce", OP.add,
                    replica_groups=[list(range(NCORES))],
                    ins=[st_in[:].opt()], outs=[st_out[:].opt()])
                stats = sbB.tile([P, 2], f32, tag="stats")
                nc.sync.dma_start(stats[:], st_out[:])
                transpose_row(bnp[:, 0:1],
                              rows_sb[4][:, 256 * layer:256 * layer + P])
                transpose_row(bnp[:, 1:2],
                              rows_sb[4][:, 256 * layer + P:256 * layer + 2 * P])
                mu = sbB.tile([P, 8], f32, tag="mu")
                nc.vector.tensor_scalar(out=mu[:, 0:1], in0=stats[:, 0:1],
                                        scalar1=1.0 / N, scalar2=None, op0=OP.mult)
                nc.vector.tensor_scalar(out=mu[:, 1:2], in0=stats[:, 1:2],
                                        scalar1=1.0 / N, scalar2=None, op0=OP.mult)
                # var = E[y^2] - mu^2: compute (mu*-mu) + E[y2]
                nc.vector.tensor_scalar(out=mu[:, 6:7], in0=mu[:, 0:1],
                                        scalar1=-1.0, scalar2=None, op0=OP.mult)
                nc.vector.scalar_tensor_tensor(
                    out=mu[:, 2:3], in0=mu[:, 0:1], scalar=mu[:, 6:7],
                    in1=mu[:, 1:2], op0=OP.mult, op1=OP.add)
                sd = sbB.tile([P, 2], f32, tag="sd")
                nc.scalar.activation(sd[:, 0:1], mu[:, 2:3], AF.Sqrt, bias=eps_ap)
                nc.vector.reciprocal(sd[:, 1:2], sd[:, 0:1])
                # a = gamma*rs ; b = beta - mu*a
                nc.vector.tensor_tensor(out=mu[:, 3:4], in0=bnp[:, 0:1],
                                        in1=sd[:, 1:2], op=OP.mult)
                nc.vector.scalar_tensor_tensor(
                    out=mu[:, 4:5], in0=mu[:, 0:1], scalar=mu[:, 3:4],
                    in1=bnp[:, 1:2], op0=OP.mult, op1=OP.subtract)
                nc.vector.tensor_scalar(out=mu[:, 5:6], in0=mu[:, 4:5],
                                        scalar1=-1.0, scalar2=None, op0=OP.mult)
                nc.scalar.activation(out_sb[:], yT[:],
                                     AF.Relu, bias=mu[:, 5:6], scale=mu[:, 3:4])

            # ---------------- phase L1 dense
            dense_tables(0,
                         lambda c: xw_all[c][:, 0:XP],
                         lambda: t_xw.ap()[:, 0:XP])
            # ---------------- L1 edge
            edge_phase(0, xin_dram)
            # ---------------- W1 + BN1 + relu -> hT
            xinT_sb = sb.tile([P, 3 * PER_CORE], f16, tag="xinT")
            transpose_load(xinT_sb, xin_dram)
            yT = sb.tile([P, PER_CORE], f32, tag="yT")
            hT_sb = sbB.tile([P, PER_CORE], f16, tag="featchunk")
            bn_phase(yT,
                     [xw_all[4][:, XP + kk * P:XP + (kk + 1) * P].bitcast(f16)
                      for kk in range(3)],
                     [xinT_sb[:, i * PER_CORE:(i + 1) * PER_CORE]
                      for i in range(3)],
                     0, hT_sb)
            nc.sync.dma_start(hT_bounce[:], hT_sb[:])
            nc.gpsimd.collective_compute(
                "AllGather", mybir.AluOpType.bypass,
                replica_groups=[list(range(NCORES))],
                ins=[hT_bounce[:].opt()], outs=[hT_all[:].opt()])
            # ---------------- L2 dense
            dense_tables(1,
                         lambda c: hT_all[c],
                         lambda: hT_bounce[:])
            # ---------------- L2 edge
            edge_phase(1, h2_dram)
            # ---------------- final: W2 on [h2 | x_in] + BN2 + relu
            h2T_sb = sb.tile([P, 3 * PER_CORE], f16, tag="h2T")
            transpose_load(h2T_sb, h2_dram)
            y2T = sb.tile([P, PER_CORE], f32, tag="yT")
            o16_sb = sbB.tile([P, PER_CORE], f16, tag="o16", bufs=1)
            w2_srcs = ([xw_all[5][:, XP + kk * P:XP + (kk + 1) * P].bitcast(f16)
                        for kk in range(4)] +
                       [xw_all[6][:, XP + kk * P:XP + (kk + 1) * P].bitcast(f16)
                        for kk in range(2)])
            bn_phase(y2T, w2_srcs,
                     [h2T_sb[:, i * PER_CORE:(i + 1) * PER_CORE]
                      for i in range(3)] +
                     [xinT_sb[:, i * PER_CORE:(i + 1) * PER_CORE]
                      for i in range(3)],
                     1, o16_sb)
            # int8 quantization with per-partition (=channel) scale
            rmax = sbB.tile([P, 4], f32, tag="rmax")
            nc.vector.tensor_reduce(out=rmax[:, 0:1], in_=o16_sb[:],
                                    axis=mybir.AxisListType.X, op=OP.max)
            nc.vector.tensor_scalar(out=rmax[:, 1:2], in0=rmax[:, 0:1],
                                    scalar1=1e-6, scalar2=None, op0=OP.max)
            nc.vector.reciprocal(rmax[:, 2:3], rmax[:, 1:2])
            nc.vector.tensor_scalar(out=rmax[:, 3:4], in0=rmax[:, 2:3],
                                    scalar1=QF, scalar2=None, op0=OP.mult)
            qt = sbB.tile([P, PER_CORE + 4], i8, tag="qt", bufs=1)
            nc.vector.tensor_scalar(out=qt[:, 0:PER_CORE], in0=o16_sb[:],
                                    scalar1=rmax[:, 3:4], scalar2=None,
                                    op0=OP.mult)
            sc = sbB.tile([P, 1], f32, tag="sc")
            nc.vector.tensor_scalar(out=sc[:], in0=rmax[:, 1:2],
                                    scalar1=1.0 / QF, scalar2=None, op0=OP.mult)
            nc.vector.tensor_copy(qt[:, PER_CORE:PER_CORE + 4],
                                  sc[:].bitcast(i8))
            nc.sync.dma_start(t_out.ap(), qt[:])

    nc.compile()
    return nc


# ------------------------------------------------------------- cached runner
def _build_runner(nc):
    import jax
    import jax.numpy as jnp
    from jax.sharding import Mesh, PartitionSpec, NamedSharding
    from jax.experimental.shard_map import shard_map
    import concourse.mybir as mybir
    from concourse.bass2jax import (_bass_exec_p, partition_id_tensor,
                                    install_neuronx_cc_hook)

    install_neuronx_cc_hook()
    partition_name = (nc.partition_id_tensor.name
                      if nc.partition_id_tensor else None)
    in_names, out_names, out_avals = [], [], []
    for alloc in nc.m.functions[0].allocations:
        if not isinstance(alloc, mybir.MemoryLocationSet):
            continue
        name = alloc.memorylocations[0].name
        if alloc.kind == "ExternalInput":
            if name != partition_name:
                in_names.append(name)
        elif alloc.kind == "ExternalOutput":
            out_avals.append(jax.core.ShapedArray(tuple(alloc.tensor_shape),
                                                  mybir.dt.np(alloc.dtype)))
            out_names.append(name)
    n_params = len(in_names)
    n_outs = len(out_avals)
    in_names_all = in_names + out_names + (
        [partition_name] if partition_name else [])

    def _body(*args):
        operands = list(args)
        if partition_name is not None:
            operands.append(partition_id_tensor())
        return tuple(_bass_exec_p.bind(
            *operands, out_avals=tuple(out_avals),
            in_names=tuple(in_names_all), out_names=tuple(out_names),
            lowering_input_output_aliases=(), sim_require_finite=True,
            sim_require_nnan=True, nc=nc))

    mesh = Mesh(np.asarray(jax.devices()[:NCORES]), ("core",))
    sharding = NamedSharding(mesh, PartitionSpec("core"))
    donate = tuple(range(n_params, n_params + n_outs))
    sharded = jax.jit(
        shard_map(_body, mesh=mesh,
                  in_specs=(PartitionSpec("core"),) * (n_params + n_outs),
                  out_specs=(PartitionSpec("core"),) * n_outs,
                  check_rep=False),
        donate_argnums=donate, keep_unused=True)
    zshapes = [(NCORES * a.shape[0], *a.shape[1:]) for a in out_avals]
    zdtypes = [a.dtype for a in out_avals]
    make_zeros = jax.jit(
        lambda: tuple(jnp.zeros(s, d) for s, d in zip(zshapes, zdtypes)),
        out_shardings=tuple(sharding for _ in zshapes))

    def run(in_map_concat):
        """in_map_concat: name -> concatenated-along-axis0 np array."""
        zs = make_zeros()          # device-side; overlaps the input upload
        dev_in = [jax.device_put(in_map_concat[name], sharding)
                  for name in in_names]
        out_arrs = sharded(*dev_in, *zs)
        return {name: np.asarray(out_arrs[i])
                for i, name in enumerate(out_names)}

    return run


# ----------------------------------------------------------------- kernel()
def kernel(**inputs):
    part = _build_partition(np.asarray(inputs["edge_index"]))
    fw = _fold_weights(inputs)
    perm, K, idx = part["perm"], part["K"], part["idx"]

    key = tuple(int(k) for k in K)
    if key not in _BUILD_CACHE:
        _BUILD_CACHE[key] = _build_program(key)
    nc = _BUILD_CACHE[key]
    if key not in _RUNNER_CACHE:
        _RUNNER_CACHE[key] = _build_runner(nc)
    run = _RUNNER_CACHE[key]

    x = np.asarray(inputs["x"], np.float32)
    xpad = np.zeros((NPAD, D), np.float32)
    real = perm >= 0
    xpad[real] = x[perm[real]]
    am = max(float(np.abs(x).max()), 1e-30)
    xq = np.round(xpad.T * (2047.0 / am)).astype(np.int16)   # [128, NPAD]
    hi8 = (xq >> 4).astype(np.int8)
    nib = (xq & 15).astype(np.uint8)
    shares = _weight_shares(fw, inputs, am / 2047.0)

    IDXC = part["tot_slots"] // 128
    BLOBW = XP + WS + IDXC
    blob = np.empty((NCORES * P, BLOBW), np.int16)
    for c in range(NCORES):
        b = blob[c * P:(c + 1) * P]
        sl = slice(c * PER_CORE, (c + 1) * PER_CORE)
        nib_c = nib[:, sl]
        packed = np.ascontiguousarray(np.concatenate(
            [hi8[:, sl].view(np.uint8),
             nib_c[:, 0::2] | (nib_c[:, 1::2] << 4)], axis=1))
        b[:, 0:XP] = packed.view(np.int16)
        b[:, XP:XP + WS] = shares[c].view(np.int16)
        b[:, XP + WS:] = _wrap_idx(idx[c]).reshape(P, IDXC)

    import time as _time
    _t0 = _time.time()
    res = run({"xw": blob})
    kernel._last_run_s = _time.time() - _t0

    o = res["outT"].reshape(NCORES, P, PER_CORE + 4)
    q = o[:, :, 0:PER_CORE].astype(np.float32)
    sc = np.ascontiguousarray(o[:, :, PER_CORE:]).view(np.float32)  # [8, P, 1]
    oT = q * sc
    out = np.zeros((N, D), np.float32)
    for c in range(NCORES):
        sl = slice(c * PER_CORE, (c + 1) * PER_CORE)
        rr = real[sl]
        out[perm[sl][rr]] = oT[c].T[rr]
    return out


if __name__ == "__main__":
    import time
    data = np.load("/root/problem/inputs_cache.npy", allow_pickle=True).item()
    expected = np.load("/root/problem/expected_cache.npy")
    t0 = time.time()
    out = kernel(**data)
    print(f"kernel() took {time.time()-t0:.1f}s")
    err = np.abs(out - expected)
    am = np.abs(expected).max()
    print(f"max_abs_err={err.max():.6f} absmax={am:.4f} rel={err.max()/am:.2e}")
    for _ in range(3):
        t0 = time.time()
        kernel(**data)
        print(f"repeat: {time.time()-t0:.2f}s (run {kernel._last_run_s:.3f}s)")


# revision 37
# speedup vs baseline: 1.7099x; 1.0815x over previous
"""Distributed GATv2 (2-layer + BN/MLP) Bass kernel for 8 Trainium2 NeuronCores.

Self-contained: host-side graph partitioning/weight-folding + Bass/Tile device
program + SPMD run + output assembly.

Algorithm notes (validated against reference in numpy to ~1e-3 of absmax):
- Nodes (in-degree sorted, round-robin dealt) -> 8 cores x 3200 slots
  (3125 real + 75 pad); per-core 25 tiles of 128 dst nodes; per tile a
  degree-grid of K_t edge slots per node (K_t identical across cores).
- Per layer, each core computes the full fp16 table
  xl_ext[n] = [SCALE*w ⊙ (x@Wl)[n] | SCALE*c1*(att_h.(x@Wl)_h) | 0-pad]  (512 cols)
  (w = att weights folded with sign into Wl columns) and gathers rows by edge
  slot via dma_gather.  Z = xl_ext[src] + xr_ext[dst] (xr broadcast over k).
- score*SCALE = Z_lin[h] + sum_d (c2*sign(w_d))*|Z_d|  (leaky_relu identity:
  sum w*lrelu(z) = c1*sum(w*z) + c2*sum(sign(w)*|w*z|)).
- ex = exp(score + SHIFT) unnormalized; out = (sum_k ex*Z)/sum_k ex - xr
  (valid since sum alpha = 1), accumulated on the PE via identity-matmuls of
  ex-scaled values; per-column factor SCALE*w undone inside W1/W2 on host.
- b1/b2/bc1/bc2 vanish inside BatchNorm (constant rows).  BN stats via
  channel-major matmuls + AllReduce; h AllGather between layers.

Transfer layout (the axon tunnel ~30-60 MB/s serialized link is the
bottleneck, so the host ships the minimum): per core ONE int16 blob
[128, 2400+512+tot/128] = [12-bit-packed x shard (3200 hi-bytes + 1600
nibble pairs; dequant step folded into the layer-1 weights) | 1/8 of
the fp16 weight payload | the 16-wrapped edge-gather index grid].  The
device AllGathers the blobs (int16 dtype: the collective byte-mangles
non-fp16 bit patterns of REMOTE shards if declared f16) so every core
sees all x chunks and the full weight payload; replicated row vectors
(biases, att signs, BN params) are broadcast/transposed on the PE from
single rows; the output returns as int8 with per-channel scales packed
into 4 trailing bytes per partition; output zero-buffers are created
on-device and donated.  The jit(shard_map) dispatch is built once and
cached.  Fixed axon RPC latency is ~78 ms exec + ~80 ms fetch.
"""
import numpy as np

N = 25000
E = 400000
D = 128
H = 3
HD = H * D
ROW = 512
NEG_SLOPE = 0.2
BN_EPS = 1e-5
NCORES = 8
PER_CORE = 3200
NTILES = 25
NPAD = NCORES * PER_CORE
SCALE = 256.0
EXP_SHIFT = -8.0
C1 = (1.0 + NEG_SLOPE) / 2.0
C2 = (1.0 - NEG_SLOPE) / 2.0
SENT_LIN = -30000.0
P = 128
WS = 512                      # weight-share columns appended to each x shard
QF = 126.45                   # int8 output quant factor (overflow-safe)
XP = 2000                     # packed-x columns (10-bit: 3200 hi8 + 800 lo2)

_BUILD_CACHE = {}
_RUNNER_CACHE = {}


# ----------------------------------------------------------------- host prep
def _build_partition(edge_index):
    src = np.asarray(edge_index[0], np.int64)
    dst = np.asarray(edge_index[1], np.int64)
    deg = np.bincount(dst, minlength=N) + 1
    order = np.argsort(-deg, kind="stable")

    perm = np.full(NPAD, -1, dtype=np.int64)
    node2slot = np.empty(N, dtype=np.int64)
    for c in range(NCORES):
        nodes_c = order[c::NCORES]
        slots = c * PER_CORE + np.arange(len(nodes_c))
        perm[slots] = nodes_c
        node2slot[nodes_c] = slots

    deg_pad = np.ones(NPAD, dtype=np.int64)
    real = perm >= 0
    deg_pad[real] = deg[perm[real]]
    dp = deg_pad.reshape(NCORES, NTILES, 128)
    K = dp.max(axis=(0, 2))
    off_t = np.concatenate([[0], np.cumsum(K * 128)]).astype(np.int64)
    tot_slots = int(off_t[-1])

    SENT = NPAD
    idx = np.full((NCORES, tot_slots), SENT, dtype=np.int32)
    src_slot = node2slot[src]
    dst_slot = node2slot[dst]
    o = np.argsort(dst_slot, kind="stable")
    ss, ds_ = src_slot[o], dst_slot[o]
    gs = np.searchsorted(ds_, np.arange(NPAD), side="left")
    # edge k-position within its dst group (self loop appended at k=deg-1)
    kpos = np.arange(len(ds_)) - gs[ds_]
    all_dst = np.concatenate([ds_, np.arange(NPAD)])           # + self loops
    all_src = np.concatenate([ss, np.arange(NPAD)])
    all_k = np.concatenate([kpos, deg_pad - 1])
    cc, local = np.divmod(all_dst, PER_CORE)
    tt, pp = np.divmod(local, 128)
    flat = off_t[tt] + all_k * 128 + pp
    idx[cc, flat] = all_src
    return dict(perm=perm, K=K, idx=idx, off_t=off_t, tot_slots=tot_slots)


def _fold_weights(inputs):
    out = {}
    for layer, (wl, bl, wr, br, att) in enumerate(
        [(inputs["Wl1"], inputs["bl1"], inputs["Wr1"], inputs["br1"], inputs["att1"]),
         (inputs["Wl2"], inputs["bl2"], inputs["Wr2"], inputs["br2"], inputs["att2"])], 1):
        wl = np.asarray(wl, np.float32); bl = np.asarray(bl, np.float32)
        wr = np.asarray(wr, np.float32); br = np.asarray(br, np.float32)
        att = np.asarray(att, np.float32)
        w = att.reshape(HD)
        Din = wl.shape[0]
        wl_ext = np.zeros((Din, ROW), np.float32)
        wr_ext = np.zeros((Din, ROW), np.float32)
        bias_ext = np.zeros(ROW, np.float32)
        wl_ext[:, :HD] = wl * (SCALE * w)[None, :]
        wr_ext[:, :HD] = wr * (SCALE * w)[None, :]
        for h in range(H):
            cols = slice(h * D, (h + 1) * D)
            wl_ext[:, HD + h] = C1 * SCALE * (wl[:, cols] @ w[cols])
            wr_ext[:, HD + h] = C1 * SCALE * (wr[:, cols] @ w[cols])
        bias_ext[:HD] = (bl + br) * (SCALE * w)
        for h in range(H):
            cols = slice(h * D, (h + 1) * D)
            bias_ext[HD + h] = C1 * SCALE * ((bl[cols] + br[cols]) @ w[cols])
        out[f"wl_ext{layer}"] = wl_ext
        out[f"wr_ext{layer}"] = wr_ext
        out[f"bias_ext{layer}"] = bias_ext
        out[f"sgn{layer}"] = (C2 * np.sign(w)).astype(np.float32)
        out[f"wscale{layer}"] = SCALE * w
    out["W1_eff"] = np.asarray(inputs["W1"], np.float32) / out["wscale1"][:, None]
    W2 = np.asarray(inputs["W2"], np.float32).copy()
    W2[:HD] = W2[:HD] / out["wscale2"][:, None]
    W2[HD:] = W2[HD:] / out["wscale1"][:, None]
    out["W2_eff"] = W2
    return out


def _wrap_idx(idx_core):
    """[tot_slots] int32 -> [16, tot_slots//16] int16 (16-wrapped)."""
    return idx_core.reshape(-1, 16).T.astype(np.int16)


def _weight_shares(fw, inputs, xscale):
    """Pack the replicated weight payload into 8 [128, WS] fp16 shares.
    Share c rides in core c's blob; AllGather reconstructs all of them.
    xscale (the 12-bit x dequant step) is folded into the layer-1 W."""
    f16 = np.float16
    shares = np.zeros((NCORES, P, WS), f16)
    shares[0] = (fw["wl_ext1"] * xscale).astype(f16)
    shares[1] = (fw["wr_ext1"] * xscale).astype(f16)
    shares[2] = fw["wl_ext2"].astype(f16)
    shares[3] = fw["wr_ext2"].astype(f16)
    W1c = fw["W1_eff"].reshape(3, P, P)
    shares[4][:, 0:384] = W1c.transpose(1, 0, 2).reshape(P, 384).astype(f16)
    shares[4][:, 384:512] = np.eye(P, dtype=f16)
    W2c = fw["W2_eff"].reshape(6, P, P)
    shares[5] = W2c[0:4].transpose(1, 0, 2).reshape(P, 512).astype(f16)
    shares[6][:, 0:256] = W2c[4:6].transpose(1, 0, 2).reshape(P, 256).astype(f16)
    # rows: p0 bias1, p1 bias2, p2 sgn1, p3 sgn2, p4 [g1|be1|g2|be2]
    shares[7][0, :] = fw["bias_ext1"].astype(f16)
    shares[7][1, :] = fw["bias_ext2"].astype(f16)
    shares[7][2, 0:HD] = fw["sgn1"].astype(f16)
    shares[7][3, 0:HD] = fw["sgn2"].astype(f16)
    shares[7][4, :] = np.concatenate(
        [np.asarray(inputs["g1"], np.float32), np.asarray(inputs["be1"], np.float32),
         np.asarray(inputs["g2"], np.float32), np.asarray(inputs["be2"], np.float32)]
    ).astype(f16)
    return shares


# ------------------------------------------------------------- device build
def _build_program(K_tuple):
    import concourse.bass as bass
    import concourse.mybir as mybir
    import concourse.tile as tile
    from concourse import bacc

    K = list(K_tuple)
    off_t = np.concatenate([[0], np.cumsum(np.array(K) * 128)]).astype(np.int64)
    tot_slots = int(off_t[-1])
    KMAX = max(K)
    IDXC = tot_slots // 128          # idx columns inside the blob (f16-sized)
    BLOBW = XP + WS + IDXC
    f16, f32, i16 = mybir.dt.float16, mybir.dt.float32, mybir.dt.int16
    i8, u8 = mybir.dt.int8, mybir.dt.uint8
    AF = mybir.ActivationFunctionType
    OP = mybir.AluOpType

    nc = bacc.Bacc("TRN2", target_bir_lowering=False, debug=False,
                   num_devices=NCORES)

    def const_col(val, dtype=f32):
        t = nc.alloc_sbuf_tensor(f"cc-{val}", [P, 1], dtype)
        nc.gpsimd.memset(t.ap(), float(val))
        nc.const_aps.aps[(dtype, float(val))] = t.ap()
        return t.ap()

    shift_ap = const_col(EXP_SHIFT)
    eps_ap = const_col(BN_EPS)
    nc.all_engine_barrier()

    # ---- inputs
    def din(name, shape, dt):
        return nc.dram_tensor(name, shape, dt, kind="ExternalInput")

    t_xw = din("xw", [P, BLOBW], i16)        # [packed x_own | w share | idx]
    t_out = nc.dram_tensor("outT", [P, PER_CORE + 4], i8, kind="ExternalOutput")

    with tile.TileContext(nc) as tc:
        with tc.tile_pool(name="sb", bufs=1) as sb, \
             tc.tile_pool(name="sbB", bufs=2) as sbB, \
             tc.tile_pool(name="sbB3", bufs=2) as sbB3, \
             tc.tile_pool(name="junkp", bufs=4) as junkp, \
             tc.tile_pool(name="psum", bufs=2, space="PSUM") as psp, \
             tc.tile_pool(name="psumD", bufs=4, space="PSUM") as pspD, \
             tc.tile_pool(name="dram", bufs=1, space="DRAM") as dram:

            # dram scratch
            xw_bounce = dram.tile([P, BLOBW], i16, tag="xwb")
            xw_all = dram.tile([NCORES, P, BLOBW], i16, tag="xwall")
            xl_tab = dram.tile([NPAD + P, ROW], f16, tag="xl_tab")
            xin_dram = dram.tile([PER_CORE, HD], f16, tag="xin")
            h2_dram = dram.tile([PER_CORE, HD], f16, tag="h2")
            hT_bounce = dram.tile([P, PER_CORE], f16, tag="hTb")
            hT_all = dram.tile([NCORES, P, PER_CORE], f16, tag="hTall")
            st_in = dram.tile([P, 2], f32, tag="st_in")
            st_out = dram.tile([P, 2], f32, tag="st_out")

            # ---- AllGather the blobs (bounce via SBUF into internal DRAM)
            xw_sb = sbB.tile([P, BLOBW], i16, tag="xwsb", bufs=1)
            nc.sync.dma_start(xw_sb[:], t_xw.ap())
            nc.sync.dma_start(xw_bounce[:], xw_sb[:])
            nc.gpsimd.collective_compute(
                "AllGather", mybir.AluOpType.bypass,
                replica_groups=[list(range(NCORES))],
                ins=[xw_bounce[:].opt()], outs=[xw_all[:].opt()])

            # ---- resident small tensors
            idx_sb = sb.tile([P, tot_slots // 16], i16, tag="idx")
            idx_src = (t_xw.ap()[:, XP + WS:XP + WS + IDXC]
                       .rearrange("(s a) c -> s a c", a=8))
            for g in range(8):
                nc.sync.dma_start(
                    idx_sb[16 * g:16 * (g + 1), :]
                    .rearrange("p (a c) -> p a c", a=8),
                    idx_src)
            ones_row = sb.tile([1, P], f16, tag="ones_row")
            nc.gpsimd.memset(ones_row[:], 1.0)
            rows_sb = []
            for r in range(5):
                row_r = sb.tile([1, ROW], f16, tag=f"row{r}", name=f"row{r}")
                rows_sb.append(row_r)
            for r in range(5):
                nc.sync.dma_start(rows_sb[r][:],
                                  xw_all[7][r:r + 1, XP:XP + ROW].bitcast(f16))
            I_sb = sb.tile([P, P], f16, tag="ident")
            nc.sync.dma_start(I_sb[:],
                              xw_all[4][:, XP + 384:XP + 512].bitcast(f16))
            wl_sb = sb.tile([P, ROW], f16, tag="wl")
            wr_sb = sb.tile([P, ROW], f16, tag="wr")
            bias_sb = sb.tile([P, ROW], f16, tag="bias")
            sgn_sb = sb.tile([P, HD], f16, tag="sgn")
            xr_all = sb.tile([P, NTILES * ROW], f16, tag="xr_all")
            bnp = sb.tile([P, 2], f32, tag="bnp")

            def bcast_row(dst, row_ap, ncols):
                """dst[:, 0:ncols] (f16 sbuf) = broadcast of row_ap [1, ncols]."""
                ps = pspD.tile([P, ROW], f32, tag="psD")
                nc.tensor.matmul(ps[:, 0:ncols], ones_row[:], row_ap,
                                 start=True, stop=True)
                nc.vector.tensor_copy(dst[:, 0:ncols], ps[:, 0:ncols])

            def transpose_row(dst_col, row_ap):
                """dst_col [P, 1] f32 sbuf = row_ap [1, P] transposed."""
                ps = pspD.tile([P, ROW], f32, tag="psD")
                nc.tensor.matmul(ps[:, 0:1], row_ap, ones_row[:, 0:1],
                                 start=True, stop=True)
                nc.vector.tensor_copy(dst_col, ps[:, 0:1])

            def unpack_x(fc, src_ap):
                """fc [P, PER_CORE] f16 <- 10-bit-packed x ints from src_ap
                ([P, XP] i16-typed region: 3200 hi-bytes then 800 bytes of
                four 2-bit lows each).  Values come out as exact integers in
                [-512, 511]; the dequant step is folded into the layer-1
                weights.  Note: the tile dep-tracker misses reads through
                size-changing bitcast APs, so stage is a native-u8 tile and
                the one i8 view read is sandwiched between tracked native
                reads on the DVE."""
                Q = PER_CORE // 4                                # 800
                stage = sbB.tile([P, 2 * XP], u8, tag="xstage", bufs=1)
                nc.sync.dma_start(stage[:], src_ap.bitcast(u8))
                hi = stage[:, 0:PER_CORE].bitcast(i8)            # [P, 3200]
                lo = stage[:, PER_CORE:2 * XP]                   # [P, 800] u8
                q0 = sbB3.tile([P, Q], u8, tag="q0", bufs=1)
                nc.vector.tensor_scalar(out=q0[:], in0=lo, scalar1=3,
                                        scalar2=None, op0=OP.bitwise_and)
                nc.vector.tensor_scalar(out=fc[:], in0=hi, scalar1=4.0,
                                        scalar2=None, op0=OP.mult)
                qs = [q0]
                for i in (1, 2, 3):
                    qi = sbB3.tile([P, Q], u8, tag=f"q{i}", bufs=1,
                                   name=f"q{i}")
                    nc.vector.tensor_scalar(out=qi[:], in0=lo, scalar1=2 * i,
                                            scalar2=3,
                                            op0=OP.logical_shift_right,
                                            op1=OP.bitwise_and)
                    qs.append(qi)
                fcv = fc[:].rearrange("p (c t) -> p c t", t=4)
                for i in range(4):
                    qf = sbB3.tile([P, Q], f16, tag=f"qf{i}", bufs=1,
                                   name=f"qf{i}")
                    nc.vector.tensor_copy(qf[:], qs[i][:])
                    nc.vector.tensor_tensor(out=fcv[:, :, i:i + 1],
                                            in0=fcv[:, :, i:i + 1],
                                            in1=qf[:, :, None], op=OP.add)

            def dense_tables(layer, chunk_src, own_src):
                """Write xl table (all nodes) + xr_all (own shard) for layer.
                chunk_src(c)/own_src() -> DRAM AP for node chunk c / own
                shard: packed [P, XP] for layer 0, plain [P, PER_CORE] f16
                for layer 1."""
                nc.sync.dma_start(wl_sb[:],
                                  xw_all[2 * layer][:, XP:XP + ROW].bitcast(f16))
                nc.sync.dma_start(wr_sb[:],
                                  xw_all[2 * layer + 1][:, XP:XP + ROW].bitcast(f16))
                bcast_row(bias_sb, rows_sb[layer][:], ROW)
                bcast_row(sgn_sb, rows_sb[2 + layer][:, 0:HD], HD)
                for c in range(NCORES):
                    fc = sbB.tile([P, PER_CORE], f16, tag="featchunk")
                    if layer == 0:
                        unpack_x(fc, chunk_src(c))
                    else:
                        nc.sync.dma_start(fc[:], chunk_src(c))
                    for tt in range(NTILES):
                        t = c * NTILES + tt
                        ps = pspD.tile([P, ROW], f32, tag="psD")
                        nc.tensor.matmul(ps[:], fc[:, tt * P:(tt + 1) * P],
                                         wl_sb[:], start=True, stop=True)
                        ot = sbB3.tile([P, ROW], f16, tag="xlrow")
                        if t % 2 == 0:
                            nc.scalar.copy(ot[:], ps[:])
                        else:
                            nc.vector.tensor_copy(ot[:], ps[:])
                        nc.sync.dma_start(xl_tab[t * P:(t + 1) * P, :], ot[:])
                # sentinel rows: zeros except big-negative linear-score cols
                sent_sb = sbB.tile([P, ROW], f16, tag="sentsb")
                nc.gpsimd.memset(sent_sb[:], 0.0)
                nc.gpsimd.memset(sent_sb[:, HD:HD + H], SENT_LIN)
                nc.sync.dma_start(xl_tab[NPAD:NPAD + P, :], sent_sb[:])
                oc = sbB.tile([P, PER_CORE], f16, tag="featchunk")
                if layer == 0:
                    unpack_x(oc, own_src())
                else:
                    nc.sync.dma_start(oc[:], own_src())
                for t in range(NTILES):
                    ps = pspD.tile([P, ROW], f32, tag="psD")
                    nc.tensor.matmul(ps[:], oc[:, t * P:(t + 1) * P],
                                     wr_sb[:], start=True, stop=True)
                    nc.vector.tensor_tensor(
                        out=xr_all[:, t * ROW:(t + 1) * ROW],
                        in0=ps[:], in1=bias_sb[:], op=OP.add)

            def edge_phase(layer, out_dram):
                for t in range(NTILES):
                    kt = K[t]
                    gb = sbB.tile([P, KMAX, ROW], f16, tag="gbufA", bufs=1)
                    o16 = int(off_t[t]) // 16
                    for kc in range(0, kt, 8):
                        nk = min(8, kt - kc)
                        nc.gpsimd.dma_gather(
                            out_ap=gb[:, kc:kc + nk, :],
                            in_ap=xl_tab[:],
                            idxs_ap=idx_sb[:, o16 + kc * 8:o16 + (kc + nk) * 8],
                            num_idxs=nk * P,
                            num_idxs_reg=nk * P,
                            elem_size=ROW,
                        )
                    xr_t = xr_all[:, t * ROW:t * ROW + 388]
                    nc.vector.tensor_tensor(
                        out=gb[:, 0:kt, 0:388], in0=gb[:, 0:kt, 0:388],
                        in1=xr_t[:, None, :].to_broadcast([P, kt, 388]),
                        op=OP.add)
                    sacc = sbB.tile([P, KMAX, 4], f32, tag="sacc")
                    for k in range(kt):
                        ab = sbB3.tile([P, HD], f16, tag="abs")
                        nc.scalar.activation(ab[:], gb[:, k, 0:HD], AF.Abs)
                        for h in range(H):
                            jt = junkp.tile([P, P], f16, tag="junk")
                            nc.vector.scalar_tensor_tensor(
                                out=jt[:],
                                in0=ab[:, h * P:(h + 1) * P],
                                scalar=1.0,
                                in1=sgn_sb[:, h * P:(h + 1) * P],
                                op0=OP.mult, op1=OP.mult,
                                accum_out=sacc[:, k, h:h + 1])
                    nc.vector.tensor_tensor(
                        out=sacc[:, 0:kt, 0:3], in0=sacc[:, 0:kt, 0:3],
                        in1=gb[:, 0:kt, HD:HD + 3], op=OP.add)
                    ex = sbB.tile([P, KMAX, 4], f32, tag="ex")
                    nc.scalar.activation(ex[:, 0:kt, 0:3], sacc[:, 0:kt, 0:3],
                                         AF.Exp, bias=shift_ap,
                                         scale=1.0 / SCALE)
                    den = sbB.tile([P, 4], f32, tag="den")
                    nc.vector.tensor_reduce(
                        out=den[:, 0:3],
                        in_=ex[:, 0:kt, 0:3].rearrange("p k h -> p h k"),
                        axis=mybir.AxisListType.X, op=OP.add)
                    denr = sbB.tile([P, 4], f32, tag="denr")
                    nc.vector.reciprocal(denr[:, 0:3], den[:, 0:3])
                    po = psp.tile([P, HD], f32, tag="pout")
                    for k in range(kt):
                        xls = sbB3.tile([P, HD], f16, tag="xls")
                        for h in range(H):
                            nc.vector.tensor_scalar(
                                out=xls[:, h * P:(h + 1) * P],
                                in0=gb[:, k, h * P:(h + 1) * P],
                                scalar1=ex[:, k, h:h + 1], scalar2=None,
                                op0=OP.mult)
                        nc.tensor.matmul(po[:], I_sb[:], xls[:],
                                         start=(k == 0), stop=(k == kt - 1))
                    xo = sbB3.tile([P, HD], f16, tag="xout")
                    for h in range(H):
                        nc.vector.scalar_tensor_tensor(
                            out=xo[:, h * P:(h + 1) * P],
                            in0=po[:, h * P:(h + 1) * P],
                            scalar=denr[:, h:h + 1],
                            in1=xr_all[:, t * ROW + h * P:t * ROW + (h + 1) * P],
                            op0=OP.mult, op1=OP.subtract)
                    nc.sync.dma_start(out_dram[t * P:(t + 1) * P, :], xo[:])

            def transpose_load(dst_sb, src_dram):
                for c3 in range(3):
                    nc.sync.dma_start_transpose(
                        dst_sb[:, c3 * PER_CORE:(c3 + 1) * PER_CORE],
                        src_dram[:, c3 * P:(c3 + 1) * P])

            def bn_phase(yT, wc_srcs, rhs_list, layer, out_sb):
                """yT [P, PER_CORE] f32 <- sum_chunks Wc.T @ rhs; BN + relu."""
                nchunks = len(wc_srcs)
                Wc_sb = sb.tile([P, nchunks, P], f16, tag=f"wc{nchunks}")
                for kk in range(nchunks):
                    nc.sync.dma_start(Wc_sb[:, kk, :], wc_srcs[kk])
                NCH = (PER_CORE + 511) // 512
                for nci in range(NCH):
                    n0 = nci * 512
                    n1 = min(PER_CORE, n0 + 512)
                    ps = pspD.tile([P, 512], f32, tag="psD")
                    for kk in range(nchunks):
                        rhs = rhs_list[kk]
                        nc.tensor.matmul(ps[:, 0:n1 - n0],
                                         Wc_sb[:, kk, :],
                                         rhs[:, n0:n1],
                                         start=(kk == 0), stop=(kk == nchunks - 1))
                    if nci % 2 == 0:
                        nc.scalar.copy(yT[:, n0:n1], ps[:, 0:n1 - n0])
                    else:
                        nc.vector.tensor_copy(yT[:, n0:n1], ps[:, 0:n1 - n0])
                nc.gpsimd.memset(yT[:, PER_CORE - 75:], 0.0)
                ssum = sbB.tile([P, 2], f32, tag="ssum")
                nc.vector.tensor_reduce(out=ssum[:, 0:1], in_=yT[:],
                                        axis=mybir.AxisListType.X, op=OP.add)
                sqj = sb.tile([P, 3 * PER_CORE], f16, tag="h2T")
                nc.scalar.activation(sqj[:, 0:PER_CORE], yT[:], AF.Square,
                                     accum_out=ssum[:, 1:2])
                nc.sync.dma_start(st_in[:], ssum[:])
                nc.gpsimd.collective_compute(
                    "AllReduce", OP.add,
                    replica_groups=[list(range(NCORES))],
                    ins=[st_in[:].opt()], outs=[st_out[:].opt()])
                stats = sbB.tile([P, 2], f32, tag="stats")
                nc.sync.dma_start(stats[:], st_out[:])
                transpose_row(bnp[:, 0:1],
                              rows_sb[4][:, 256 * layer:256 * layer + P])
                transpose_row(bnp[:, 1:2],
                              rows_sb[4][:, 256 * layer + P:256 * layer + 2 * P])
                mu = sbB.tile([P, 8], f32, tag="mu")
                nc.vector.tensor_scalar(out=mu[:, 0:1], in0=stats[:, 0:1],
                                        scalar1=1.0 / N, scalar2=None, op0=OP.mult)
                nc.vector.tensor_scalar(out=mu[:, 1:2], in0=stats[:, 1:2],
                                        scalar1=1.0 / N, scalar2=None, op0=OP.mult)
                # var = E[y^2] - mu^2: compute (mu*-mu) + E[y2]
                nc.vector.tensor_scalar(out=mu[:, 6:7], in0=mu[:, 0:1],
                                        scalar1=-1.0, scalar2=None, op0=OP.mult)
                nc.vector.scalar_tensor_tensor(
                    out=mu[:, 2:3], in0=mu[:, 0:1], scalar=mu[:, 6:7],
                    in1=mu[:, 1:2], op0=OP.mult, op1=OP.add)
                sd = sbB.tile([P, 2], f32, tag="sd")
                nc.scalar.activation(sd[:, 0:1], mu[:, 2:3], AF.Sqrt, bias=eps_ap)
                nc.vector.reciprocal(sd[:, 1:2], sd[:, 0:1])
                # a = gamma*rs ; b = beta - mu*a
                nc.vector.tensor_tensor(out=mu[:, 3:4], in0=bnp[:, 0:1],
                                        in1=sd[:, 1:2], op=OP.mult)
                nc.vector.scalar_tensor_tensor(
                    out=mu[:, 4:5], in0=mu[:, 0:1], scalar=mu[:, 3:4],
                    in1=bnp[:, 1:2], op0=OP.mult, op1=OP.subtract)
                nc.vector.tensor_scalar(out=mu[:, 5:6], in0=mu[:, 4:5],
                                        scalar1=-1.0, scalar2=None, op0=OP.mult)
                nc.scalar.activation(out_sb[:], yT[:],
                                     AF.Relu, bias=mu[:, 5:6], scale=mu[:, 3:4])

            # ---------------- phase L1 dense
            dense_tables(0,
                         lambda c: xw_all[c][:, 0:XP],
                         lambda: t_xw.ap()[:, 0:XP])
            # ---------------- L1 edge
            edge_phase(0, xin_dram)
            # ---------------- W1 + BN1 + relu -> hT
            xinT_sb = sb.tile([P, 3 * PER_CORE], f16, tag="xinT")
            transpose_load(xinT_sb, xin_dram)
            yT = sb.tile([P, PER_CORE], f32, tag="yT")
            hT_sb = sbB.tile([P, PER_CORE], f16, tag="featchunk")
            bn_phase(yT,
                     [xw_all[4][:, XP + kk * P:XP + (kk + 1) * P].bitcast(f16)
                      for kk in range(3)],
                     [xinT_sb[:, i * PER_CORE:(i + 1) * PER_CORE]
                      for i in range(3)],
                     0, hT_sb)
            nc.sync.dma_start(hT_bounce[:], hT_sb[:])
            nc.gpsimd.collective_compute(
                "AllGather", mybir.AluOpType.bypass,
                replica_groups=[list(range(NCORES))],
                ins=[hT_bounce[:].opt()], outs=[hT_all[:].opt()])
            # ---------------- L2 dense
            dense_tables(1,
                         lambda c: hT_all[c],
                         lambda: hT_bounce[:])
            # ---------------- L2 edge
            edge_phase(1, h2_dram)
            # ---------------- final: W2 on [h2 | x_in] + BN2 + relu
            h2T_sb = sb.tile([P, 3 * PER_CORE], f16, tag="h2T")
            transpose_load(h2T_sb, h2_dram)
            y2T = sb.tile([P, PER_CORE], f32, tag="yT")
            o16_sb = sbB.tile([P, PER_CORE], f16, tag="o16", bufs=1)
            w2_srcs = ([xw_all[5][:, XP + kk * P:XP + (kk + 1) * P].bitcast(f16)
                        for kk in range(4)] +
                       [xw_all[6][:, XP + kk * P:XP + (kk + 1) * P].bitcast(f16)
                        for kk in range(2)])
            bn_phase(y2T, w2_srcs,
                     [h2T_sb[:, i * PER_CORE:(i + 1) * PER_CORE]
                      for i in range(3)] +
                     [xinT_sb[:, i * PER_CORE:(i + 1) * PER_CORE]
                      for i in range(3)],
                     1, o16_sb)
            # int8 quantization with per-partition (=channel) scale
            rmax = sbB.tile([P, 4], f32, tag="rmax")
            nc.vector.tensor_reduce(out=rmax[:, 0:1], in_=o16_sb[:],
                                    axis=mybir.AxisListType.X, op=OP.max)
            nc.vector.tensor_scalar(out=rmax[:, 1:2], in0=rmax[:, 0:1],
                                    scalar1=1e-6, scalar2=None, op0=OP.max)
            nc.vector.reciprocal(rmax[:, 2:3], rmax[:, 1:2])
            nc.vector.tensor_scalar(out=rmax[:, 3:4], in0=rmax[:, 2:3],
                                    scalar1=QF, scalar2=None, op0=OP.mult)
            qt = sbB.tile([P, PER_CORE + 4], i8, tag="qt", bufs=1)
            nc.vector.tensor_scalar(out=qt[:, 0:PER_CORE], in0=o16_sb[:],
                                    scalar1=rmax[:, 3:4], scalar2=None,
                                    op0=OP.mult)
            sc = sbB.tile([P, 1], f32, tag="sc")
            nc.vector.tensor_scalar(out=sc[:], in0=rmax[:, 1:2],
                                    scalar1=1.0 / QF, scalar2=None, op0=OP.mult)
            nc.vector.tensor_copy(qt[:, PER_CORE:PER_CORE + 4],
                                  sc[:].bitcast(i8))
            nc.sync.dma_start(t_out.ap(), qt[:])

    nc.compile()
    return nc


# ------------------------------------------------------------- cached runner
def _build_runner(nc):
    import jax
    import jax.numpy as jnp
    from jax.sharding import Mesh, PartitionSpec, NamedSharding
    from jax.experimental.shard_map import shard_map
    import concourse.mybir as mybir
    from concourse.bass2jax import (_bass_exec_p, partition_id_tensor,
                                    install_neuronx_cc_hook)

    install_neuronx_cc_hook()
    partition_name = (nc.partition_id_tensor.name
                      if nc.partition_id_tensor else None)
    in_names, out_names, out_avals = [], [], []
    for alloc in nc.m.functions[0].allocations:
        if not isinstance(alloc, mybir.MemoryLocationSet):
            continue
        name = alloc.memorylocations[0].name
        if alloc.kind == "ExternalInput":
            if name != partition_name:
                in_names.append(name)
        elif alloc.kind == "ExternalOutput":
            out_avals.append(jax.core.ShapedArray(tuple(alloc.tensor_shape),
                                                  mybir.dt.np(alloc.dtype)))
            out_names.append(name)
    n_params = len(in_names)
    n_outs = len(out_avals)
    in_names_all = in_names + out_names + (
        [partition_name] if partition_name else [])

    def _body(*args):
        operands = list(args)
        if partition_name is not None:
            operands.append(partition_id_tensor())
        return tuple(_bass_exec_p.bind(
            *operands, out_avals=tuple(out_avals),
            in_names=tuple(in_names_all), out_names=tuple(out_names),
            lowering_input_output_aliases=(), sim_require_finite=True,
            sim_require_nnan=True, nc=nc))

    mesh = Mesh(np.asarray(jax.devices()[:NCORES]), ("core",))
    sharding = NamedSharding(mesh, PartitionSpec("core"))
    donate = tuple(range(n_params, n_params + n_outs))
    sharded = jax.jit(
        shard_map(_body, mesh=mesh,
                  in_specs=(PartitionSpec("core"),) * (n_params + n_outs),
                  out_specs=(PartitionSpec("core"),) * n_outs,
                  check_rep=False),
        donate_argnums=donate, keep_unused=True)
    zshapes = [(NCORES * a.shape[0], *a.shape[1:]) for a in out_avals]
    zdtypes = [a.dtype for a in out_avals]
    make_zeros = jax.jit(
        lambda: tuple(jnp.zeros(s, d) for s, d in zip(zshapes, zdtypes)),
        out_shardings=tuple(sharding for _ in zshapes))

    def run(in_map_concat):
        """in_map_concat: name -> concatenated-along-axis0 np array."""
        zs = make_zeros()          # device-side; overlaps the input upload
        dev_in = [jax.device_put(in_map_concat[name], sharding)
                  for name in in_names]
        out_arrs = sharded(*dev_in, *zs)
        return {name: np.asarray(out_arrs[i])
                for i, name in enumerate(out_names)}

    return run


# ----------------------------------------------------------------- kernel()
def kernel(**inputs):
    part = _build_partition(np.asarray(inputs["edge_index"]))
    fw = _fold_weights(inputs)
    perm, K, idx = part["perm"], part["K"], part["idx"]

    key = tuple(int(k) for k in K)
    if key not in _BUILD_CACHE:
        _BUILD_CACHE[key] = _build_program(key)
    nc = _BUILD_CACHE[key]
    if key not in _RUNNER_CACHE:
        _RUNNER_CACHE[key] = _build_runner(nc)
    run = _RUNNER_CACHE[key]

    x = np.asarray(inputs["x"], np.float32)
    xpad = np.zeros((NPAD, D), np.float32)
    real = perm >= 0
    xpad[real] = x[perm[real]]
    am = max(float(np.abs(x).max()), 1e-30)
    xq = np.round(xpad.T * (511.0 / am)).astype(np.int16)    # [128, NPAD]
    hi8 = (xq >> 2).astype(np.int8)
    lo2 = (xq & 3).astype(np.uint8)
    shares = _weight_shares(fw, inputs, am / 511.0)

    IDXC = part["tot_slots"] // 128
    BLOBW = XP + WS + IDXC
    blob = np.empty((NCORES * P, BLOBW), np.int16)
    for c in range(NCORES):
        b = blob[c * P:(c + 1) * P]
        sl = slice(c * PER_CORE, (c + 1) * PER_CORE)
        lo_c = lo2[:, sl]
        packed = np.ascontiguousarray(np.concatenate(
            [hi8[:, sl].view(np.uint8),
             lo_c[:, 0::4] | (lo_c[:, 1::4] << 2) |
             (lo_c[:, 2::4] << 4) | (lo_c[:, 3::4] << 6)], axis=1))
        b[:, 0:XP] = packed.view(np.int16)
        b[:, XP:XP + WS] = shares[c].view(np.int16)
        b[:, XP + WS:] = _wrap_idx(idx[c]).reshape(P, IDXC)

    import time as _time
    _t0 = _time.time()
    res = run({"xw": blob})
    kernel._last_run_s = _time.time() - _t0

    o = res["outT"].reshape(NCORES, P, PER_CORE + 4)
    q = o[:, :, 0:PER_CORE].astype(np.float32)
    sc = np.ascontiguousarray(o[:, :, PER_CORE:]).view(np.float32)  # [8, P, 1]
    oT = q * sc
    out = np.zeros((N, D), np.float32)
    for c in range(NCORES):
        sl = slice(c * PER_CORE, (c + 1) * PER_CORE)
        rr = real[sl]
        out[perm[sl][rr]] = oT[c].T[rr]
    return out


if __name__ == "__main__":
    import time
    data = np.load("/root/problem/inputs_cache.npy", allow_pickle=True).item()
    expected = np.load("/root/problem/expected_cache.npy")
    t0 = time.time()
    out = kernel(**data)
    print(f"kernel() took {time.time()-t0:.1f}s")
    err = np.abs(out - expected)
    am = np.abs(expected).max()
    print(f"max_abs_err={err.max():.6f} absmax={am:.4f} rel={err.max()/am:.2e}")
    for _ in range(3):
        t0 = time.time()
        kernel(**data)
        print(f"repeat: {time.time()-t0:.2f}s (run {kernel._last_run_s:.3f}s)")


# revision 39
# speedup vs baseline: 1.7154x; 1.0032x over previous
"""Distributed GATv2 (2-layer + BN/MLP) Bass kernel for 8 Trainium2 NeuronCores.

Self-contained: host-side graph partitioning/weight-folding + Bass/Tile device
program + SPMD run + output assembly.

Algorithm notes (validated against reference in numpy to ~1e-3 of absmax):
- Nodes (in-degree sorted, round-robin dealt) -> 8 cores x 3200 slots
  (3125 real + 75 pad); per-core 25 tiles of 128 dst nodes; per tile a
  degree-grid of K_t edge slots per node (K_t identical across cores).
- Per layer, each core computes the full fp16 table
  xl_ext[n] = [SCALE*w ⊙ (x@Wl)[n] | SCALE*c1*(att_h.(x@Wl)_h) | 0-pad]  (512 cols)
  (w = att weights folded with sign into Wl columns) and gathers rows by edge
  slot via dma_gather.  Z = xl_ext[src] + xr_ext[dst] (xr broadcast over k).
- score*SCALE = Z_lin[h] + sum_d (c2*sign(w_d))*|Z_d|  (leaky_relu identity:
  sum w*lrelu(z) = c1*sum(w*z) + c2*sum(sign(w)*|w*z|)).
- ex = exp(score + SHIFT) unnormalized; out = (sum_k ex*Z)/sum_k ex - xr
  (valid since sum alpha = 1), accumulated on the PE via identity-matmuls of
  ex-scaled values; per-column factor SCALE*w undone inside W1/W2 on host.
- b1/b2/bc1/bc2 vanish inside BatchNorm (constant rows).  BN stats via
  channel-major matmuls + AllReduce; h AllGather between layers.

Transfer layout (the axon tunnel ~30-60 MB/s serialized link is the
bottleneck, so the host ships the minimum): per core ONE int16 blob
[128, 2000+512+tot/128] = [10-bit-packed x shard (3200 hi-bytes + 800
2-bit-low bytes; dequant step folded into the layer-1 weights) | 1/8 of
the fp16 weight payload | the 16-wrapped edge-gather index grid].  The
device AllGathers the blobs (int16 dtype: the collective byte-mangles
non-fp16 bit patterns of REMOTE shards if declared f16) so every core
sees all x chunks and the full weight payload; replicated row vectors
(biases, att signs, BN params) are broadcast/transposed on the PE from
single rows; the output returns as int8 with per-channel scales packed
into 4 trailing bytes per partition; output zero-buffers are created
on-device and donated.  The jit(shard_map) dispatch is built once and
cached.  Fixed axon RPC latency is ~78 ms exec + ~80 ms fetch.
"""
import numpy as np

N = 25000
E = 400000
D = 128
H = 3
HD = H * D
ROW = 512
NEG_SLOPE = 0.2
BN_EPS = 1e-5
NCORES = 8
PER_CORE = 3200
NTILES = 25
NPAD = NCORES * PER_CORE
SCALE = 256.0
EXP_SHIFT = -8.0
C1 = (1.0 + NEG_SLOPE) / 2.0
C2 = (1.0 - NEG_SLOPE) / 2.0
SENT_LIN = -30000.0
P = 128
WS = 512                      # weight-share columns appended to each x shard
QF = 126.45                   # int8 output quant factor (overflow-safe)
XP = 2000                     # packed-x columns (10-bit: 3200 hi8 + 800 lo2)

_BUILD_CACHE = {}
_RUNNER_CACHE = {}


# ----------------------------------------------------------------- host prep
def _build_partition(edge_index):
    src = np.asarray(edge_index[0], np.int64)
    dst = np.asarray(edge_index[1], np.int64)
    deg = np.bincount(dst, minlength=N) + 1
    order = np.argsort(-deg, kind="stable")

    perm = np.full(NPAD, -1, dtype=np.int64)
    node2slot = np.empty(N, dtype=np.int64)
    for c in range(NCORES):
        nodes_c = order[c::NCORES]
        slots = c * PER_CORE + np.arange(len(nodes_c))
        perm[slots] = nodes_c
        node2slot[nodes_c] = slots

    deg_pad = np.ones(NPAD, dtype=np.int64)
    real = perm >= 0
    deg_pad[real] = deg[perm[real]]
    dp = deg_pad.reshape(NCORES, NTILES, 128)
    K = dp.max(axis=(0, 2))
    off_t = np.concatenate([[0], np.cumsum(K * 128)]).astype(np.int64)
    tot_slots = int(off_t[-1])

    SENT = NPAD
    idx = np.full((NCORES, tot_slots), SENT, dtype=np.int32)
    src_slot = node2slot[src]
    dst_slot = node2slot[dst]
    o = np.argsort(dst_slot, kind="stable")
    ss, ds_ = src_slot[o], dst_slot[o]
    gs = np.searchsorted(ds_, np.arange(NPAD), side="left")
    # edge k-position within its dst group (self loop appended at k=deg-1)
    kpos = np.arange(len(ds_)) - gs[ds_]
    all_dst = np.concatenate([ds_, np.arange(NPAD)])           # + self loops
    all_src = np.concatenate([ss, np.arange(NPAD)])
    all_k = np.concatenate([kpos, deg_pad - 1])
    cc, local = np.divmod(all_dst, PER_CORE)
    tt, pp = np.divmod(local, 128)
    flat = off_t[tt] + all_k * 128 + pp
    idx[cc, flat] = all_src
    return dict(perm=perm, K=K, idx=idx, off_t=off_t, tot_slots=tot_slots)


def _fold_weights(inputs):
    out = {}
    for layer, (wl, bl, wr, br, att) in enumerate(
        [(inputs["Wl1"], inputs["bl1"], inputs["Wr1"], inputs["br1"], inputs["att1"]),
         (inputs["Wl2"], inputs["bl2"], inputs["Wr2"], inputs["br2"], inputs["att2"])], 1):
        wl = np.asarray(wl, np.float32); bl = np.asarray(bl, np.float32)
        wr = np.asarray(wr, np.float32); br = np.asarray(br, np.float32)
        att = np.asarray(att, np.float32)
        w = att.reshape(HD)
        Din = wl.shape[0]
        wl_ext = np.zeros((Din, ROW), np.float32)
        wr_ext = np.zeros((Din, ROW), np.float32)
        bias_ext = np.zeros(ROW, np.float32)
        wl_ext[:, :HD] = wl * (SCALE * w)[None, :]
        wr_ext[:, :HD] = wr * (SCALE * w)[None, :]
        for h in range(H):
            cols = slice(h * D, (h + 1) * D)
            wl_ext[:, HD + h] = C1 * SCALE * (wl[:, cols] @ w[cols])
            wr_ext[:, HD + h] = C1 * SCALE * (wr[:, cols] @ w[cols])
        bias_ext[:HD] = (bl + br) * (SCALE * w)
        for h in range(H):
            cols = slice(h * D, (h + 1) * D)
            bias_ext[HD + h] = C1 * SCALE * ((bl[cols] + br[cols]) @ w[cols])
        out[f"wl_ext{layer}"] = wl_ext
        out[f"wr_ext{layer}"] = wr_ext
        out[f"bias_ext{layer}"] = bias_ext
        out[f"sgn{layer}"] = (C2 * np.sign(w)).astype(np.float32)
        out[f"wscale{layer}"] = SCALE * w
    out["W1_eff"] = np.asarray(inputs["W1"], np.float32) / out["wscale1"][:, None]
    W2 = np.asarray(inputs["W2"], np.float32).copy()
    W2[:HD] = W2[:HD] / out["wscale2"][:, None]
    W2[HD:] = W2[HD:] / out["wscale1"][:, None]
    out["W2_eff"] = W2
    return out


def _wrap_idx(idx_core):
    """[tot_slots] int32 -> [16, tot_slots//16] int16 (16-wrapped)."""
    return idx_core.reshape(-1, 16).T.astype(np.int16)


def _weight_shares(fw, inputs, xscale):
    """Pack the replicated weight payload into 8 [128, WS] fp16 shares.
    Share c rides in core c's blob; AllGather reconstructs all of them.
    xscale (the packed-x dequant step) is folded into the layer-1 W."""
    f16 = np.float16
    shares = np.zeros((NCORES, P, WS), f16)
    shares[0] = (fw["wl_ext1"] * xscale).astype(f16)
    shares[1] = (fw["wr_ext1"] * xscale).astype(f16)
    shares[2] = fw["wl_ext2"].astype(f16)
    shares[3] = fw["wr_ext2"].astype(f16)
    W1c = fw["W1_eff"].reshape(3, P, P)
    shares[4][:, 0:384] = W1c.transpose(1, 0, 2).reshape(P, 384).astype(f16)
    shares[4][:, 384:512] = np.eye(P, dtype=f16)
    W2c = fw["W2_eff"].reshape(6, P, P)
    shares[5] = W2c[0:4].transpose(1, 0, 2).reshape(P, 512).astype(f16)
    shares[6][:, 0:256] = W2c[4:6].transpose(1, 0, 2).reshape(P, 256).astype(f16)
    # rows: p0 bias1, p1 bias2, p2 sgn1, p3 sgn2, p4 [g1|be1|g2|be2]
    shares[7][0, :] = fw["bias_ext1"].astype(f16)
    shares[7][1, :] = fw["bias_ext2"].astype(f16)
    shares[7][2, 0:HD] = fw["sgn1"].astype(f16)
    shares[7][3, 0:HD] = fw["sgn2"].astype(f16)
    shares[7][4, :] = np.concatenate(
        [np.asarray(inputs["g1"], np.float32), np.asarray(inputs["be1"], np.float32),
         np.asarray(inputs["g2"], np.float32), np.asarray(inputs["be2"], np.float32)]
    ).astype(f16)
    return shares


# ------------------------------------------------------------- device build
def _build_program(K_tuple):
    import concourse.bass as bass
    import concourse.mybir as mybir
    import concourse.tile as tile
    from concourse import bacc

    K = list(K_tuple)
    off_t = np.concatenate([[0], np.cumsum(np.array(K) * 128)]).astype(np.int64)
    tot_slots = int(off_t[-1])
    KMAX = max(K)
    IDXC = tot_slots // 128          # idx columns inside the blob (f16-sized)
    BLOBW = XP + WS + IDXC
    f16, f32, i16 = mybir.dt.float16, mybir.dt.float32, mybir.dt.int16
    i8, u8 = mybir.dt.int8, mybir.dt.uint8
    AF = mybir.ActivationFunctionType
    OP = mybir.AluOpType

    nc = bacc.Bacc("TRN2", target_bir_lowering=False, debug=False,
                   num_devices=NCORES)

    def const_col(val, dtype=f32):
        t = nc.alloc_sbuf_tensor(f"cc-{val}", [P, 1], dtype)
        nc.gpsimd.memset(t.ap(), float(val))
        nc.const_aps.aps[(dtype, float(val))] = t.ap()
        return t.ap()

    shift_ap = const_col(EXP_SHIFT)
    eps_ap = const_col(BN_EPS)
    nc.all_engine_barrier()

    # ---- inputs
    def din(name, shape, dt):
        return nc.dram_tensor(name, shape, dt, kind="ExternalInput")

    t_xw = din("xw", [P, BLOBW], i16)        # [packed x_own | w share | idx]
    t_out = nc.dram_tensor("outT", [P, PER_CORE + 4], i8, kind="ExternalOutput")

    with tile.TileContext(nc) as tc:
        with tc.tile_pool(name="sb", bufs=1) as sb, \
             tc.tile_pool(name="sbB", bufs=2) as sbB, \
             tc.tile_pool(name="sbB3", bufs=2) as sbB3, \
             tc.tile_pool(name="junkp", bufs=4) as junkp, \
             tc.tile_pool(name="psum", bufs=2, space="PSUM") as psp, \
             tc.tile_pool(name="psumD", bufs=4, space="PSUM") as pspD, \
             tc.tile_pool(name="dram", bufs=1, space="DRAM") as dram:

            # dram scratch
            xw_bounce = dram.tile([P, BLOBW], i16, tag="xwb")
            xw_all = dram.tile([NCORES, P, BLOBW], i16, tag="xwall")
            xl_tab = dram.tile([NPAD + P, ROW], f16, tag="xl_tab")
            xin_dram = dram.tile([PER_CORE, HD], f16, tag="xin")
            h2_dram = dram.tile([PER_CORE, HD], f16, tag="h2")
            hT_bounce = dram.tile([P, PER_CORE], f16, tag="hTb")
            hT_all = dram.tile([NCORES, P, PER_CORE], f16, tag="hTall")
            st_in = dram.tile([P, 2], f32, tag="st_in")
            st_out = dram.tile([P, 2], f32, tag="st_out")

            # ---- AllGather the blobs (bounce via SBUF into internal DRAM)
            xw_sb = sbB.tile([P, BLOBW], i16, tag="xwsb", bufs=1)
            nc.sync.dma_start(xw_sb[:], t_xw.ap())
            nc.sync.dma_start(xw_bounce[:], xw_sb[:])
            nc.gpsimd.collective_compute(
                "AllGather", mybir.AluOpType.bypass,
                replica_groups=[list(range(NCORES))],
                ins=[xw_bounce[:].opt()], outs=[xw_all[:].opt()])

            # ---- resident small tensors
            idx_sb = sb.tile([P, tot_slots // 16], i16, tag="idx")
            idx_src = (t_xw.ap()[:, XP + WS:XP + WS + IDXC]
                       .rearrange("(s a) c -> s a c", a=8))
            for g in range(8):
                nc.sync.dma_start(
                    idx_sb[16 * g:16 * (g + 1), :]
                    .rearrange("p (a c) -> p a c", a=8),
                    idx_src)
            ones_row = sb.tile([1, P], f16, tag="ones_row")
            nc.gpsimd.memset(ones_row[:], 1.0)
            rows_sb = []
            for r in range(5):
                row_r = sb.tile([1, ROW], f16, tag=f"row{r}", name=f"row{r}")
                rows_sb.append(row_r)
            for r in range(5):
                nc.sync.dma_start(rows_sb[r][:],
                                  xw_all[7][r:r + 1, XP:XP + ROW].bitcast(f16))
            I_sb = sb.tile([P, P], f16, tag="ident")
            nc.sync.dma_start(I_sb[:],
                              xw_all[4][:, XP + 384:XP + 512].bitcast(f16))
            wl_sb = sb.tile([P, ROW], f16, tag="wl")
            wr_sb = sb.tile([P, ROW], f16, tag="wr")
            bias_sb = sb.tile([P, ROW], f16, tag="bias")
            sgn_sb = sb.tile([P, HD], f16, tag="sgn")
            xr_all = sb.tile([P, NTILES * ROW], f16, tag="xr_all")
            bnp = sb.tile([P, 2], f32, tag="bnp")

            def bcast_row(dst, row_ap, ncols):
                """dst[:, 0:ncols] (f16 sbuf) = broadcast of row_ap [1, ncols]."""
                ps = pspD.tile([P, ROW], f32, tag="psD")
                nc.tensor.matmul(ps[:, 0:ncols], ones_row[:], row_ap,
                                 start=True, stop=True)
                nc.vector.tensor_copy(dst[:, 0:ncols], ps[:, 0:ncols])

            def transpose_row(dst_col, row_ap):
                """dst_col [P, 1] f32 sbuf = row_ap [1, P] transposed."""
                ps = pspD.tile([P, ROW], f32, tag="psD")
                nc.tensor.matmul(ps[:, 0:1], row_ap, ones_row[:, 0:1],
                                 start=True, stop=True)
                nc.vector.tensor_copy(dst_col, ps[:, 0:1])

            def unpack_x(fc, src_ap):
                """fc [P, PER_CORE] f16 <- 10-bit-packed x ints from src_ap
                ([P, XP] i16-typed region: 3200 hi-bytes then 800 bytes of
                four 2-bit lows each).  Values come out as exact integers in
                [-512, 511]; the dequant step is folded into the layer-1
                weights.  Note: the tile dep-tracker misses reads through
                size-changing bitcast APs, so stage is a native-u8 tile and
                the one i8 view read is sandwiched between tracked native
                reads on the DVE."""
                Q = PER_CORE // 4                                # 800
                stage = sbB.tile([P, 2 * XP], u8, tag="xstage", bufs=1)
                nc.sync.dma_start(stage[:], src_ap.bitcast(u8))
                hi = stage[:, 0:PER_CORE].bitcast(i8)            # [P, 3200]
                lo = stage[:, PER_CORE:2 * XP]                   # [P, 800] u8
                q0 = sbB3.tile([P, Q], u8, tag="q0", bufs=1)
                nc.vector.tensor_scalar(out=q0[:], in0=lo, scalar1=3,
                                        scalar2=None, op0=OP.bitwise_and)
                nc.vector.tensor_scalar(out=fc[:], in0=hi, scalar1=4.0,
                                        scalar2=None, op0=OP.mult)
                qs = [q0]
                for i in (1, 2, 3):
                    qi = sbB3.tile([P, Q], u8, tag=f"q{i}", bufs=1,
                                   name=f"q{i}")
                    nc.vector.tensor_scalar(out=qi[:], in0=lo, scalar1=2 * i,
                                            scalar2=3,
                                            op0=OP.logical_shift_right,
                                            op1=OP.bitwise_and)
                    qs.append(qi)
                fcv = fc[:].rearrange("p (c t) -> p c t", t=4)
                for i in range(4):
                    qf = sbB3.tile([P, Q], f16, tag=f"qf{i}", bufs=1,
                                   name=f"qf{i}")
                    nc.vector.tensor_copy(qf[:], qs[i][:])
                    nc.vector.tensor_tensor(out=fcv[:, :, i:i + 1],
                                            in0=fcv[:, :, i:i + 1],
                                            in1=qf[:, :, None], op=OP.add)

            def dense_tables(layer, chunk_src, own_src):
                """Write xl table (all nodes) + xr_all (own shard) for layer.
                chunk_src(c)/own_src() -> DRAM AP for node chunk c / own
                shard: packed [P, XP] for layer 0, plain [P, PER_CORE] f16
                for layer 1."""
                nc.sync.dma_start(wl_sb[:],
                                  xw_all[2 * layer][:, XP:XP + ROW].bitcast(f16))
                nc.sync.dma_start(wr_sb[:],
                                  xw_all[2 * layer + 1][:, XP:XP + ROW].bitcast(f16))
                bcast_row(bias_sb, rows_sb[layer][:], ROW)
                bcast_row(sgn_sb, rows_sb[2 + layer][:, 0:HD], HD)
                for c in range(NCORES):
                    fc = sbB.tile([P, PER_CORE], f16, tag="featchunk")
                    if layer == 0:
                        unpack_x(fc, chunk_src(c))
                    else:
                        nc.sync.dma_start(fc[:], chunk_src(c))
                    for tt in range(NTILES):
                        t = c * NTILES + tt
                        ps = pspD.tile([P, ROW], f32, tag="psD")
                        nc.tensor.matmul(ps[:], fc[:, tt * P:(tt + 1) * P],
                                         wl_sb[:], start=True, stop=True)
                        ot = sbB3.tile([P, ROW], f16, tag="xlrow")
                        if t % 2 == 0:
                            nc.scalar.copy(ot[:], ps[:])
                        else:
                            nc.vector.tensor_copy(ot[:], ps[:])
                        nc.sync.dma_start(xl_tab[t * P:(t + 1) * P, :], ot[:])
                # sentinel rows: zeros except big-negative linear-score cols
                sent_sb = sbB.tile([P, ROW], f16, tag="sentsb")
                nc.gpsimd.memset(sent_sb[:], 0.0)
                nc.gpsimd.memset(sent_sb[:, HD:HD + H], SENT_LIN)
                nc.sync.dma_start(xl_tab[NPAD:NPAD + P, :], sent_sb[:])
                oc = sbB.tile([P, PER_CORE], f16, tag="featchunk")
                if layer == 0:
                    unpack_x(oc, own_src())
                else:
                    nc.sync.dma_start(oc[:], own_src())
                for t in range(NTILES):
                    ps = pspD.tile([P, ROW], f32, tag="psD")
                    nc.tensor.matmul(ps[:], oc[:, t * P:(t + 1) * P],
                                     wr_sb[:], start=True, stop=True)
                    nc.vector.tensor_tensor(
                        out=xr_all[:, t * ROW:(t + 1) * ROW],
                        in0=ps[:], in1=bias_sb[:], op=OP.add)

            def edge_phase(layer, out_dram):
                for t in range(NTILES):
                    kt = K[t]
                    gb = sbB.tile([P, KMAX, ROW], f16, tag="gbufA", bufs=1)
                    o16 = int(off_t[t]) // 16
                    for kc in range(0, kt, 8):
                        nk = min(8, kt - kc)
                        nc.gpsimd.dma_gather(
                            out_ap=gb[:, kc:kc + nk, :],
                            in_ap=xl_tab[:],
                            idxs_ap=idx_sb[:, o16 + kc * 8:o16 + (kc + nk) * 8],
                            num_idxs=nk * P,
                            num_idxs_reg=nk * P,
                            elem_size=ROW,
                        )
                    xr_t = xr_all[:, t * ROW:t * ROW + 388]
                    nc.vector.tensor_tensor(
                        out=gb[:, 0:kt, 0:388], in0=gb[:, 0:kt, 0:388],
                        in1=xr_t[:, None, :].to_broadcast([P, kt, 388]),
                        op=OP.add)
                    sacc = sbB.tile([P, KMAX, 4], f32, tag="sacc")
                    for k in range(kt):
                        ab = sbB3.tile([P, HD], f16, tag="abs")
                        nc.scalar.activation(ab[:], gb[:, k, 0:HD], AF.Abs)
                        for h in range(H):
                            jt = junkp.tile([P, P], f16, tag="junk")
                            nc.vector.scalar_tensor_tensor(
                                out=jt[:],
                                in0=ab[:, h * P:(h + 1) * P],
                                scalar=1.0,
                                in1=sgn_sb[:, h * P:(h + 1) * P],
                                op0=OP.mult, op1=OP.mult,
                                accum_out=sacc[:, k, h:h + 1])
                    nc.vector.tensor_tensor(
                        out=sacc[:, 0:kt, 0:3], in0=sacc[:, 0:kt, 0:3],
                        in1=gb[:, 0:kt, HD:HD + 3], op=OP.add)
                    ex = sbB.tile([P, KMAX, 4], f32, tag="ex")
                    nc.scalar.activation(ex[:, 0:kt, 0:3], sacc[:, 0:kt, 0:3],
                                         AF.Exp, bias=shift_ap,
                                         scale=1.0 / SCALE)
                    den = sbB.tile([P, 4], f32, tag="den")
                    nc.vector.tensor_reduce(
                        out=den[:, 0:3],
                        in_=ex[:, 0:kt, 0:3].rearrange("p k h -> p h k"),
                        axis=mybir.AxisListType.X, op=OP.add)
                    denr = sbB.tile([P, 4], f32, tag="denr")
                    nc.vector.reciprocal(denr[:, 0:3], den[:, 0:3])
                    po = psp.tile([P, HD], f32, tag="pout")
                    for k in range(kt):
                        xls = sbB3.tile([P, HD], f16, tag="xls")
                        for h in range(H):
                            nc.vector.tensor_scalar(
                                out=xls[:, h * P:(h + 1) * P],
                                in0=gb[:, k, h * P:(h + 1) * P],
                                scalar1=ex[:, k, h:h + 1], scalar2=None,
                                op0=OP.mult)
                        nc.tensor.matmul(po[:], I_sb[:], xls[:],
                                         start=(k == 0), stop=(k == kt - 1))
                    xo = sbB3.tile([P, HD], f16, tag="xout")
                    for h in range(H):
                        nc.vector.scalar_tensor_tensor(
                            out=xo[:, h * P:(h + 1) * P],
                            in0=po[:, h * P:(h + 1) * P],
                            scalar=denr[:, h:h + 1],
                            in1=xr_all[:, t * ROW + h * P:t * ROW + (h + 1) * P],
                            op0=OP.mult, op1=OP.subtract)
                    nc.sync.dma_start(out_dram[t * P:(t + 1) * P, :], xo[:])

            def transpose_load(dst_sb, src_dram):
                for c3 in range(3):
                    nc.sync.dma_start_transpose(
                        dst_sb[:, c3 * PER_CORE:(c3 + 1) * PER_CORE],
                        src_dram[:, c3 * P:(c3 + 1) * P])

            def bn_phase(yT, wc_srcs, rhs_list, layer, out_sb):
                """yT [P, PER_CORE] f32 <- sum_chunks Wc.T @ rhs; BN + relu."""
                nchunks = len(wc_srcs)
                Wc_sb = sb.tile([P, nchunks, P], f16, tag=f"wc{nchunks}")
                for kk in range(nchunks):
                    nc.sync.dma_start(Wc_sb[:, kk, :], wc_srcs[kk])
                NCH = (PER_CORE + 511) // 512
                for nci in range(NCH):
                    n0 = nci * 512
                    n1 = min(PER_CORE, n0 + 512)
                    ps = pspD.tile([P, 512], f32, tag="psD")
                    for kk in range(nchunks):
                        rhs = rhs_list[kk]
                        nc.tensor.matmul(ps[:, 0:n1 - n0],
                                         Wc_sb[:, kk, :],
                                         rhs[:, n0:n1],
                                         start=(kk == 0), stop=(kk == nchunks - 1))
                    if nci % 2 == 0:
                        nc.scalar.copy(yT[:, n0:n1], ps[:, 0:n1 - n0])
                    else:
                        nc.vector.tensor_copy(yT[:, n0:n1], ps[:, 0:n1 - n0])
                nc.gpsimd.memset(yT[:, PER_CORE - 75:], 0.0)
                ssum = sbB.tile([P, 2], f32, tag="ssum")
                nc.vector.tensor_reduce(out=ssum[:, 0:1], in_=yT[:],
                                        axis=mybir.AxisListType.X, op=OP.add)
                sqj = sb.tile([P, 3 * PER_CORE], f16, tag="h2T")
                nc.scalar.activation(sqj[:, 0:PER_CORE], yT[:], AF.Square,
                                     accum_out=ssum[:, 1:2])
                nc.sync.dma_start(st_in[:], ssum[:])
                nc.gpsimd.collective_compute(
                    "AllReduce", OP.add,
                    replica_groups=[list(range(NCORES))],
                    ins=[st_in[:].opt()], outs=[st_out[:].opt()])
                stats = sbB.tile([P, 2], f32, tag="stats")
                nc.sync.dma_start(stats[:], st_out[:])
                transpose_row(bnp[:, 0:1],
                              rows_sb[4][:, 256 * layer:256 * layer + P])
                transpose_row(bnp[:, 1:2],
                              rows_sb[4][:, 256 * layer + P:256 * layer + 2 * P])
                mu = sbB.tile([P, 8], f32, tag="mu")
                nc.vector.tensor_scalar(out=mu[:, 0:1], in0=stats[:, 0:1],
                                        scalar1=1.0 / N, scalar2=None, op0=OP.mult)
                nc.vector.tensor_scalar(out=mu[:, 1:2], in0=stats[:, 1:2],
                                        scalar1=1.0 / N, scalar2=None, op0=OP.mult)
                # var = E[y^2] - mu^2: compute (mu*-mu) + E[y2]
                nc.vector.tensor_scalar(out=mu[:, 6:7], in0=mu[:, 0:1],
                                        scalar1=-1.0, scalar2=None, op0=OP.mult)
                nc.vector.scalar_tensor_tensor(
                    out=mu[:, 2:3], in0=mu[:, 0:1], scalar=mu[:, 6:7],
                    in1=mu[:, 1:2], op0=OP.mult, op1=OP.add)
                sd = sbB.tile([P, 2], f32, tag="sd")
                nc.scalar.activation(sd[:, 0:1], mu[:, 2:3], AF.Sqrt, bias=eps_ap)
                nc.vector.reciprocal(sd[:, 1:2], sd[:, 0:1])
                # a = gamma*rs ; b = beta - mu*a
                nc.vector.tensor_tensor(out=mu[:, 3:4], in0=bnp[:, 0:1],
                                        in1=sd[:, 1:2], op=OP.mult)
                nc.vector.scalar_tensor_tensor(
                    out=mu[:, 4:5], in0=mu[:, 0:1], scalar=mu[:, 3:4],
                    in1=bnp[:, 1:2], op0=OP.mult, op1=OP.subtract)
                nc.vector.tensor_scalar(out=mu[:, 5:6], in0=mu[:, 4:5],
                                        scalar1=-1.0, scalar2=None, op0=OP.mult)
                nc.scalar.activation(out_sb[:], yT[:],
                                     AF.Relu, bias=mu[:, 5:6], scale=mu[:, 3:4])

            # ---------------- phase L1 dense
            dense_tables(0,
                         lambda c: xw_all[c][:, 0:XP],
                         lambda: t_xw.ap()[:, 0:XP])
            # ---------------- L1 edge
            edge_phase(0, xin_dram)
            # ---------------- W1 + BN1 + relu -> hT
            xinT_sb = sb.tile([P, 3 * PER_CORE], f16, tag="xinT")
            transpose_load(xinT_sb, xin_dram)
            yT = sb.tile([P, PER_CORE], f32, tag="yT")
            hT_sb = sbB.tile([P, PER_CORE], f16, tag="featchunk")
            bn_phase(yT,
                     [xw_all[4][:, XP + kk * P:XP + (kk + 1) * P].bitcast(f16)
                      for kk in range(3)],
                     [xinT_sb[:, i * PER_CORE:(i + 1) * PER_CORE]
                      for i in range(3)],
                     0, hT_sb)
            nc.sync.dma_start(hT_bounce[:], hT_sb[:])
            nc.gpsimd.collective_compute(
                "AllGather", mybir.AluOpType.bypass,
                replica_groups=[list(range(NCORES))],
                ins=[hT_bounce[:].opt()], outs=[hT_all[:].opt()])
            # ---------------- L2 dense
            dense_tables(1,
                         lambda c: hT_all[c],
                         lambda: hT_bounce[:])
            # ---------------- L2 edge
            edge_phase(1, h2_dram)
            # ---------------- final: W2 on [h2 | x_in] + BN2 + relu
            h2T_sb = sb.tile([P, 3 * PER_CORE], f16, tag="h2T")
            transpose_load(h2T_sb, h2_dram)
            y2T = sb.tile([P, PER_CORE], f32, tag="yT")
            o16_sb = sbB.tile([P, PER_CORE], f16, tag="o16", bufs=1)
            w2_srcs = ([xw_all[5][:, XP + kk * P:XP + (kk + 1) * P].bitcast(f16)
                        for kk in range(4)] +
                       [xw_all[6][:, XP + kk * P:XP + (kk + 1) * P].bitcast(f16)
                        for kk in range(2)])
            bn_phase(y2T, w2_srcs,
                     [h2T_sb[:, i * PER_CORE:(i + 1) * PER_CORE]
                      for i in range(3)] +
                     [xinT_sb[:, i * PER_CORE:(i + 1) * PER_CORE]
                      for i in range(3)],
                     1, o16_sb)
            # int8 quantization with per-partition (=channel) scale
            rmax = sbB.tile([P, 4], f32, tag="rmax")
            nc.vector.tensor_reduce(out=rmax[:, 0:1], in_=o16_sb[:],
                                    axis=mybir.AxisListType.X, op=OP.max)
            nc.vector.tensor_scalar(out=rmax[:, 1:2], in0=rmax[:, 0:1],
                                    scalar1=1e-6, scalar2=None, op0=OP.max)
            nc.vector.reciprocal(rmax[:, 2:3], rmax[:, 1:2])
            nc.vector.tensor_scalar(out=rmax[:, 3:4], in0=rmax[:, 2:3],
                                    scalar1=QF, scalar2=None, op0=OP.mult)
            qt = sbB.tile([P, PER_CORE + 4], i8, tag="qt", bufs=1)
            nc.vector.tensor_scalar(out=qt[:, 0:PER_CORE], in0=o16_sb[:],
                                    scalar1=rmax[:, 3:4], scalar2=None,
                                    op0=OP.mult)
            sc = sbB.tile([P, 1], f32, tag="sc")
            nc.vector.tensor_scalar(out=sc[:], in0=rmax[:, 1:2],
                                    scalar1=1.0 / QF, scalar2=None, op0=OP.mult)
            nc.vector.tensor_copy(qt[:, PER_CORE:PER_CORE + 4],
                                  sc[:].bitcast(i8))
            nc.sync.dma_start(t_out.ap(), qt[:])

    nc.compile()
    return nc


# ------------------------------------------------------------- cached runner
def _build_runner(nc):
    import jax
    import jax.numpy as jnp
    from jax.sharding import Mesh, PartitionSpec, NamedSharding
    from jax.experimental.shard_map import shard_map
    import concourse.mybir as mybir
    from concourse.bass2jax import (_bass_exec_p, partition_id_tensor,
                                    install_neuronx_cc_hook)

    install_neuronx_cc_hook()
    partition_name = (nc.partition_id_tensor.name
                      if nc.partition_id_tensor else None)
    in_names, out_names, out_avals = [], [], []
    for alloc in nc.m.functions[0].allocations:
        if not isinstance(alloc, mybir.MemoryLocationSet):
            continue
        name = alloc.memorylocations[0].name
        if alloc.kind == "ExternalInput":
            if name != partition_name:
                in_names.append(name)
        elif alloc.kind == "ExternalOutput":
            out_avals.append(jax.core.ShapedArray(tuple(alloc.tensor_shape),
                                                  mybir.dt.np(alloc.dtype)))
            out_names.append(name)
    n_params = len(in_names)
    n_outs = len(out_avals)
    in_names_all = in_names + out_names + (
        [partition_name] if partition_name else [])

    def _body(*args):
        operands = list(args)
        if partition_name is not None:
            operands.append(partition_id_tensor())
        return tuple(_bass_exec_p.bind(
            *operands, out_avals=tuple(out_avals),
            in_names=tuple(in_names_all), out_names=tuple(out_names),
            lowering_input_output_aliases=(), sim_require_finite=True,
            sim_require_nnan=True, nc=nc))

    mesh = Mesh(np.asarray(jax.devices()[:NCORES]), ("core",))
    sharding = NamedSharding(mesh, PartitionSpec("core"))
    donate = tuple(range(n_params, n_params + n_outs))
    sharded = jax.jit(
        shard_map(_body, mesh=mesh,
                  in_specs=(PartitionSpec("core"),) * (n_params + n_outs),
                  out_specs=(PartitionSpec("core"),) * n_outs,
                  check_rep=False),
        donate_argnums=donate, keep_unused=True)
    zshapes = [(NCORES * a.shape[0], *a.shape[1:]) for a in out_avals]
    zdtypes = [a.dtype for a in out_avals]
    make_zeros = jax.jit(
        lambda: tuple(jnp.zeros(s, d) for s, d in zip(zshapes, zdtypes)),
        out_shardings=tuple(sharding for _ in zshapes))

    zpool = []                     # pre-made donated output buffers

    def run(in_map_concat):
        """in_map_concat: name -> concatenated-along-axis0 np array."""
        zs = zpool.pop() if zpool else make_zeros()
        dev_in = [jax.device_put(in_map_concat[name], sharding)
                  for name in in_names]
        out_arrs = sharded(*dev_in, *zs)
        zpool.append(make_zeros())  # next call's buffers; hides under fetch
        return {name: np.asarray(out_arrs[i])
                for i, name in enumerate(out_names)}

    return run


# ----------------------------------------------------------------- kernel()
def kernel(**inputs):
    part = _build_partition(np.asarray(inputs["edge_index"]))
    fw = _fold_weights(inputs)
    perm, K, idx = part["perm"], part["K"], part["idx"]

    key = tuple(int(k) for k in K)
    if key not in _BUILD_CACHE:
        _BUILD_CACHE[key] = _build_program(key)
    nc = _BUILD_CACHE[key]
    if key not in _RUNNER_CACHE:
        _RUNNER_CACHE[key] = _build_runner(nc)
    run = _RUNNER_CACHE[key]

    x = np.asarray(inputs["x"], np.float32)
    xpad = np.zeros((NPAD, D), np.float32)
    real = perm >= 0
    xpad[real] = x[perm[real]]
    am = max(float(np.abs(x).max()), 1e-30)
    xq = np.round(xpad.T * (511.0 / am)).astype(np.int16)    # [128, NPAD]
    hi8 = (xq >> 2).astype(np.int8)
    lo2 = (xq & 3).astype(np.uint8)
    shares = _weight_shares(fw, inputs, am / 511.0)

    IDXC = part["tot_slots"] // 128
    BLOBW = XP + WS + IDXC
    blob = np.empty((NCORES * P, BLOBW), np.int16)
    for c in range(NCORES):
        b = blob[c * P:(c + 1) * P]
        sl = slice(c * PER_CORE, (c + 1) * PER_CORE)
        lo_c = lo2[:, sl]
        packed = np.ascontiguousarray(np.concatenate(
            [hi8[:, sl].view(np.uint8),
             lo_c[:, 0::4] | (lo_c[:, 1::4] << 2) |
             (lo_c[:, 2::4] << 4) | (lo_c[:, 3::4] << 6)], axis=1))
        b[:, 0:XP] = packed.view(np.int16)
        b[:, XP:XP + WS] = shares[c].view(np.int16)
        b[:, XP + WS:] = _wrap_idx(idx[c]).reshape(P, IDXC)

    import time as _time
    _t0 = _time.time()
    res = run({"xw": blob})
    kernel._last_run_s = _time.time() - _t0

    o = res["outT"].reshape(NCORES, P, PER_CORE + 4)
    q = o[:, :, 0:PER_CORE].astype(np.float32)
    sc = np.ascontiguousarray(o[:, :, PER_CORE:]).view(np.float32)  # [8, P, 1]
    oT = q * sc
    out = np.zeros((N, D), np.float32)
    for c in range(NCORES):
        sl = slice(c * PER_CORE, (c + 1) * PER_CORE)
        rr = real[sl]
        out[perm[sl][rr]] = oT[c].T[rr]
    return out


if __name__ == "__main__":
    import time
    data = np.load("/root/problem/inputs_cache.npy", allow_pickle=True).item()
    expected = np.load("/root/problem/expected_cache.npy")
    t0 = time.time()
    out = kernel(**data)
    print(f"kernel() took {time.time()-t0:.1f}s")
    err = np.abs(out - expected)
    am = np.abs(expected).max()
    print(f"max_abs_err={err.max():.6f} absmax={am:.4f} rel={err.max()/am:.2e}")
    for _ in range(3):
        t0 = time.time()
        kernel(**data)
        print(f"repeat: {time.time()-t0:.2f}s (run {kernel._last_run_s:.3f}s)")
